# revision 35
# baseline (speedup 1.0000x reference)
"""DeepseekV2 decoder layer on 8 TRN2 NeuronCores (Bass/Tile).

Sharding: TP over heads (2/core) for q/kv_b/attention/o_proj, kv_a
token-sharded (256 tokens/core) + AllGather, TP over INTER (1024/core) for
the MLP. Chunked AllReduce after o_proj (carrying x/8 = (hidden+attn)/8,
with wo/8 and hidden/64 folded in) and chunked ReduceScatter after
down_proj.

Internal layout is feature-major ("transposed"): activations live as
[feature, token] so every matmul output feeds the next as `rhs` without any
on-device transpose. All large DRAM inputs are laid out host-side as
[128, free] partition-rows so every SBUF load is one DMA with >=2KB
contiguous runs. ht streams token-chunk-major with norm+q-proj interleaved
per chunk; MLP weights prefetch behind the attention inputs; B-chunk norm
preps run inside the attention window in the space freed by the kv_b pool.
"""

import numpy as np
import ml_dtypes

import concourse.bass as bass
import concourse.mybir as mybir
import concourse.tile as tile
from concourse import bacc
from concourse.bass_utils import run_bass_kernel_spmd

BF = ml_dtypes.bfloat16

B, S, HID = 2, 1024, 2048
T = B * S                      # 2048 tokens
H = 16
DN, DR = 128, 64
DQK = DN + DR
DV = 128
KVR = 512
INTER = 8192
EPS = 1e-6
ROPE_BASE = 10000.0
SCALING = DQK ** -0.5

NC_N = 8
HPC = H // NC_N                # 2 heads per core
FPC = INTER // NC_N            # 1024 inter per core
P = 128
HCH = HID // P                 # 16 hid chunks
TT = 4                         # token chunks of 512
TW = T // TT                   # 512
SH = T // NC_N                 # 256-token kv_a shard per core
KT = S // P                    # 8 k-tiles of 128 per batch
QT = S // TW                   # 2 q-chunks of 512 per batch
KVC = KVR // P                 # 4
AGW = KVC * SH + SH            # 1280: kva (f-major) + kpe corner block
NEG = -30000.0

f32 = mybir.dt.float32
bf16 = mybir.dt.bfloat16
ADD = mybir.AluOpType.add
MUL = mybir.AluOpType.mult
AF = mybir.ActivationFunctionType

_CACHE = {}


def _build():
    nc = bacc.Bacc("TRN2", target_bir_lowering=False, debug=False, num_devices=NC_N)
    dp = lambda n, sh, dt: nc.dram_tensor(n, sh, dt, kind="ExternalInput")
    htb = dp("htb", [TT, P, HCH * TW], bf16)        # chunk-major partition rows
    htb8 = dp("htb8", [TT, P, HCH * TW], bf16)      # hidden/8, chunk-major rows
    h64c3 = dp("h64c3", [P, HCH * TW], bf16)        # hidden/64, chunk 3 only
    htsh = dp("htsh", [P, HCH * SH], bf16)          # this core's kv_a token shard
    wq = dp("wq", [P, HCH * HPC * DQK], bf16)
    wkva = dp("wkva", [P, (KVC + 1) * HCH * P], bf16)   # f-major blocks
    wkvbn = dp("wkvbn", [P, KVC * HPC * DN], bf16)
    wkvbv = dp("wkvbv", [P, KVC * HPC * DV], bf16)
    wo = dp("wo", [P, HPC * HID], bf16)             # pre-divided by 8
    wg = dp("wg", [P, HCH * FPC], bf16)
    wu = dp("wu", [P, HCH * FPC], bf16)
    wd = dp("wd", [P, (FPC // P) * HID], bf16)
    cosf = dp("cosf", [P, T], bf16)      # rows 64:128 duplicate 0:64
    sinf = dp("sinf", [P, T], bf16)
    cossh = dp("cossh", [DR, SH], bf16)
    sinsh = dp("sinsh", [DR, SH], bf16)
    masks = dp("masks", [P, TW + 384], bf16)        # shifted-window causal mask
    out = nc.dram_tensor("o", [TT, HID // NC_N, TW], bf16, kind="ExternalOutput")
    rg = [list(range(NC_N))]

    with tile.TileContext(nc) as tc:
        with tc.tile_pool(name="const", bufs=1) as cpool, \
             tc.tile_pool(name="dram", bufs=1, space="DRAM") as dram, \
             tc.tile_pool(name="ps", bufs=1, space="PSUM") as ps, \
             tc.tile_pool(name="wrk", bufs=3) as wrk, \
             tc.tile_pool(name="row", bufs=2) as row:
            ones_col = cpool.tile([P, 1], bf16)
            nc.vector.memset(ones_col[:], 1.0)
            ones_row = cpool.tile([1, P], bf16)
            nc.vector.memset(ones_row[:], 1.0)
            epsb = cpool.tile([P, 1], f32)
            nc.vector.memset(epsb[:], EPS)
            eps64 = cpool.tile([P, 1], f32)
            nc.vector.memset(eps64[:], EPS / 64.0)

            ag_in = dram.tile([P, AGW], bf16, name="ag_in")
            ag_out = dram.tile([NC_N, P, AGW], bf16, addr_space="Shared",
                               name="ag_out")
            ar_in = [dram.tile([HID, TW], bf16, name=f"ar_in{t}") for t in range(TT)]
            ar_out = [dram.tile([HID, TW], bf16, addr_space="Shared",
                                name=f"ar_out{t}") for t in range(TT)]
            rs_in = [dram.tile([HID, TW], bf16, name=f"rs_in{t}")
                     for t in range(TT)]
            rs_out = [dram.tile([HID // NC_N, TW], bf16, name=f"rs_out{t}")
                      for t in range(TT)]

            # helper: [1,W] f32 PSUM sumsq row -> wide f32 inv-scale [P,W]
            def inv_chain(ssp, w, scale, bias, name, out=None):
                nrow = row.tile([1, TW], bf16, tag="nrow", bufs=1, name=f"nr_{name}")
                nc.scalar.copy(nrow[:, :w], ssp[:, :w])
                bcp = ps.tile([P, TW], f32, tag="big", bufs=4, name=f"bc_{name}")
                nc.tensor.matmul(bcp[:, :w], ones_row[:], nrow[:, :w],
                                 start=True, stop=True)
                sd = wrk.tile([P, TW], f32, tag="sd", bufs=1, name=f"sd_{name}")
                nc.scalar.activation(sd[:, :w], bcp[:, :w], AF.Sqrt,
                                     bias=bias, scale=scale)
                if out is None:
                    out = wrk.tile([P, TW], f32, tag="inv", bufs=2,
                                   name=f"inv_{name}")[:, :w]
                nc.vector.reciprocal(out, sd[:, :w])
                return out

            with tc.tile_pool(name="pers", bufs=1) as pers:
                qsb = pers.tile([P, 3, T], bf16)           # 12K
                qrope = pers.tile([P, T], bf16)            # head h at rows h*64
                krope = pers.tile([P, T], bf16)            # both halves identical

                # ---- A0: kv_a shard + AllGather (fires early) ----
                with tc.tile_pool(name="shp", bufs=1) as shp:
                    hts = shp.tile([P, HCH, SH], bf16)
                    nc.sync.dma_start(hts[:], htsh.ap())
                    wkva_sb = shp.tile([P, KVC + 1, HCH, P], bf16)   # 20K
                    for f in range(KVC + 1):
                        nc.sync.dma_start(wkva_sb[:, f, :, :],
                                          wkva.ap()[:, f * HCH * P:(f + 1) * HCH * P])
                    css = shp.tile([DR, SH], bf16)
                    nc.sync.dma_start(css[:], cossh.ap())
                    sns = shp.tile([DR, SH], bf16)
                    nc.sync.dma_start(sns[:], sinsh.ap())

                    # shard input-norm scale r1_sh
                    sshp = ps.tile([1, TW], f32, tag="r", bufs=2, name="sshp")
                    for o in range(HCH):
                        sqs = wrk.tile([P, TW], bf16, tag="sq", bufs=2, name="sqs")
                        nc.scalar.square(sqs[:, :SH], hts[:, o, :])
                        nc.tensor.matmul(sshp[:, :SH], ones_col[:], sqs[:, :SH],
                                         start=(o == 0), stop=(o == HCH - 1))
                    rsh = inv_chain(sshp, SH, 1.0 / HID, epsb[:], "rsh")

                    # latent projection for the shard
                    lats = shp.tile([P, KVC * SH], bf16)
                    kpes = shp.tile([DR, SH], bf16)
                    ss2p = ps.tile([1, TW], f32, tag="r", bufs=2, name="ss2p")
                    for f in range(KVC + 1):
                        wid = P if f < KVC else DR
                        lp = ps.tile([P, TW], f32, tag="big", bufs=4, name="lp")
                        for o in range(HCH):
                            nc.tensor.matmul(lp[:wid, :SH],
                                             wkva_sb[:, f, o, :wid],
                                             hts[:, o, :],
                                             start=(o == 0), stop=(o == HCH - 1))
                        if f < KVC:
                            nc.vector.tensor_copy(out=lats[:, f * SH:(f + 1) * SH],
                                                  in_=lp[:, :SH])
                            sq2 = wrk.tile([P, TW], bf16, tag="sq", bufs=2, name="sq2")
                            nc.scalar.square(sq2[:, :SH], lp[:, :SH])
                            nc.tensor.matmul(ss2p[:, :SH], ones_col[:], sq2[:, :SH],
                                             start=(f == 0), stop=(f == KVC - 1))
                        else:
                            nc.vector.tensor_tensor(kpes[:], lp[:DR, :SH],
                                                    rsh[:DR, :SH], MUL)
                    r2sh = inv_chain(ss2p, SH, 1.0 / KVR, epsb[:], "r2sh")
                    kvas = shp.tile([P, KVC * SH], bf16)
                    for f in range(KVC):
                        nc.vector.tensor_tensor(kvas[:, f * SH:(f + 1) * SH],
                                                lats[:, f * SH:(f + 1) * SH],
                                                r2sh[:, :SH], MUL)
                    # rope k_pe shard: [x1(32); x2(32)] layout
                    ksw = wrk.tile([DR, SH], bf16, tag="rps", bufs=1, name="ksw")
                    nc.sync.dma_start(ksw[0:32, :], kpes[32:64, :])
                    nc.sync.dma_start(ksw[32:64, :], kpes[0:32, :])
                    ktmp = wrk.tile([DR, SH], bf16, tag="rps", bufs=1, name="ktmp")
                    nc.vector.tensor_tensor(ktmp[:], kpes[:], css[:], MUL)
                    krs = shp.tile([DR, SH], bf16)
                    nc.vector.tensor_tensor(krs[:], ksw[:], sns[:], MUL)
                    nc.vector.tensor_tensor(krs[:], krs[:], ktmp[:], ADD)
                    # pack shard -> ag_in and AllGather
                    nc.sync.dma_start(ag_in[:, 0:KVC * SH], kvas[:])
                    nc.sync.dma_start(ag_in[0:DR, KVC * SH:KVC * SH + SH], krs[:])
                    nc.gpsimd.collective_compute(
                        "AllGather", mybir.AluOpType.bypass, ins=[ag_in[:].opt()],
                        outs=[ag_out[:].opt()], replica_groups=rg)

                # ---- A1: full input norm + q projection + q rope ----
                with tc.tile_pool(name="a1", bufs=1) as a1:
                    wq_sb = a1.tile([P, HCH, HPC * DQK], bf16)   # 12K
                    nc.sync.dma_start(wq_sb[:], wq.ap())
                    ht = a1.tile([P, TT, HCH, TW], bf16)         # 64K
                    for t in range(TT):
                        for g in range(4):
                            nc.sync.dma_start(
                                ht[:, t, 4 * g:4 * g + 4, :],
                                htb.ap()[t, :, 4 * g * TW:(4 * g + 4) * TW])
                    cs = a1.tile([P, T], bf16)
                    nc.sync.dma_start(cs[:], cosf.ap())
                    sn = a1.tile([P, T], bf16)
                    nc.sync.dma_start(sn[:], sinf.ap())
                    bc1 = a1.tile([P, TT, TW], f32)              # 8K

                    for t in range(TT):
                        ssp = ps.tile([1, TW], f32, tag="r", bufs=2, name="ssp")
                        for o in range(HCH):
                            sq = wrk.tile([P, TW], bf16, tag="sq", bufs=2, name="sq")
                            nc.scalar.square(sq[:], ht[:, t, o, :])
                            nc.tensor.matmul(ssp[:], ones_col[:], sq[:],
                                             start=(o == 0), stop=(o == HCH - 1))
                        inv_chain(ssp, TW, 1.0 / HID, epsb[:], f"r1_{t}",
                                  out=bc1[:, t, :])
                        # q projection for this chunk (SCALING folded into wq)
                        for f in range(3):
                            qp = ps.tile([P, TW], f32, tag="big", bufs=4, name="qp")
                            for o in range(HCH):
                                nc.tensor.matmul(qp[:], wq_sb[:, o, f * P:(f + 1) * P],
                                                 ht[:, t, o, :],
                                                 start=(o == 0), stop=(o == HCH - 1))
                            nc.vector.tensor_tensor(qsb[:, f, t * TW:(t + 1) * TW],
                                                    qp[:], bc1[:, t, :], MUL)

                    # q rope: [x1(32); x2(32)] per head, head h on rows h*64
                    src = qsb[:, 2, :]
                    sw = a1.tile([P, T], bf16, tag="rope", bufs=2, name="qsw")
                    for h in range(HPC):
                        nc.sync.dma_start(sw[h * DR:h * DR + 32, :],
                                          src[h * DR + 32:h * DR + 64, :])
                        nc.sync.dma_start(sw[h * DR + 32:h * DR + 64, :],
                                          src[h * DR:h * DR + 32, :])
                    tmp = a1.tile([P, T], bf16, tag="rope", bufs=2, name="qtmp")
                    nc.vector.tensor_tensor(tmp[:], src, cs[:], MUL)
                    nc.vector.tensor_tensor(qrope[:], sw[:], sn[:], MUL)
                    nc.vector.tensor_tensor(qrope[:], qrope[:], tmp[:], ADD)
                    # k rope full comes pre-roped from the AllGather (both halves)
                    for c in range(NC_N):
                        for h in range(HPC):
                            nc.sync.dma_start(
                                krope[h * DR:(h + 1) * DR, c * SH:(c + 1) * SH],
                                ag_out[c, 0:DR, KVC * SH:KVC * SH + SH])

                # ---- bw: MLP weights (DMAs issued after attention inputs) ----
                with tc.tile_pool(name="bw", bufs=1) as bw:
                    wg_sb = bw.tile([P, HCH, FPC], bf16)       # 32K
                    wu_sb = bw.tile([P, HCH, FPC], bf16)       # 32K
                    wd_sb = bw.tile([P, FPC // P, HID], bf16)  # 16K

                    with tc.tile_pool(name="att", bufs=1) as att:
                        knope = att.tile([P, HPC, T], bf16)        # 8K
                        vnat = att.tile([P, T // P, HPC * DV], bf16)  # 8K
                        wo_sb = att.tile([P, HPC, HID], bf16)      # 8K
                        msk = att.tile([P, TW + 384], bf16)        # 1.75K
                        nc.sync.dma_start(msk[:], masks.ap())
                        nc.sync.dma_start(wo_sb[:], wo.ap())

                        with tc.tile_pool(name="kvp", bufs=1) as kvp:
                            # kva laid out core-major so each load is contiguous
                            kva = kvp.tile([P, NC_N, KVC, SH], bf16)   # 16K
                            for c in range(NC_N):
                                nc.sync.dma_start(kva[:, c, :, :],
                                                  ag_out[c, :, 0:KVC * SH])
                            wkvbn_sb = kvp.tile([P, KVC, HPC * DN], bf16)
                            nc.sync.dma_start(wkvbn_sb[:], wkvbn.ap())
                            wkvbv_sb = kvp.tile([P, KVC, HPC * DV], bf16)
                            nc.sync.dma_start(wkvbv_sb[:], wkvbv.ap())
                            # now queue the MLP weight prefetch behind these
                            nc.sync.dma_start(wg_sb[:], wg.ap())
                            nc.sync.dma_start(wu_sb[:], wu.ap())
                            nc.sync.dma_start(wd_sb[:], wd.ap())

                            for h in range(HPC):
                                for t in range(TT):
                                    kp = ps.tile([P, TW], f32, tag="big", bufs=4,
                                                 name="kp")
                                    for c in range(KVC):
                                        nc.tensor.matmul(
                                            kp[:], wkvbn_sb[:, c, h * P:(h + 1) * P],
                                            kva[:, 2 * t:2 * t + 2, c, :],
                                            start=(c == 0), stop=(c == KVC - 1))
                                    nc.vector.tensor_copy(
                                        out=knope[:, h, t * TW:(t + 1) * TW], in_=kp[:])
                            for to in range(T // P):
                                vp = ps.tile([P, TW], f32, tag="att", bufs=2,
                                             name="vp")
                                co, po = to // 2, (to % 2) * P
                                for c in range(KVC):
                                    nc.tensor.matmul(
                                        vp[:, :HPC * DV],
                                        kva[:, co, c, po:po + P],
                                        wkvbv_sb[:, c, :],
                                        start=(c == 0), stop=(c == KVC - 1))
                                nc.vector.tensor_copy(out=vnat[:, to, :],
                                                      in_=vp[:, :HPC * DV])

                        # B-chunk tiles live in the space kvp just freed
                        with tc.tile_pool(name="bp", bufs=1) as bp:
                            h2s = [None] * TT
                            acts = [None] * TT

                            def b_pre(t):
                                # x/8 = hidden/8 + ar_out[t] (attn/8); h2 doubles
                                # as the x/8 store. Fat chunk DMA, no AR dep.
                                h2 = bp.tile([P, HCH, TW], bf16, tag="h2", bufs=2,
                                             name=f"h2_{t}")
                                for g in range(4):
                                    nc.sync.dma_start(
                                        h2[:, 4 * g:4 * g + 4, :],
                                        htb8.ap()[t, :, 4 * g * TW:(4 * g + 4) * TW])
                                h2s[t] = h2

                            def b_pre3():
                                h2s[3] = bp.tile([P, HCH, TW], bf16, tag="h2",
                                                 bufs=2, name="h2_3")

                            def b_fin(t, folded=False):
                                # folded: ar_out[t] already holds x/8
                                h2 = h2s[t]
                                ssp3 = ps.tile([1, TW], f32, tag="r", bufs=2,
                                               name="ssp3")
                                for o in range(HCH):
                                    if folded:
                                        nc.sync.dma_start(
                                            h2[:, o, :],
                                            ar_out[t][o * P:(o + 1) * P, :])
                                    else:
                                        aro = wrk.tile([P, TW], bf16, tag="h64",
                                                       bufs=3, name="aro")
                                        nc.sync.dma_start(
                                            aro[:], ar_out[t][o * P:(o + 1) * P, :])
                                        nc.vector.tensor_tensor(h2[:, o, :],
                                                                h2[:, o, :],
                                                                aro[:], ADD)
                                    sq3 = wrk.tile([P, TW], bf16, tag="sq", bufs=2,
                                                   name="sq3")
                                    nc.scalar.square(sq3[:], h2[:, o, :])
                                    nc.tensor.matmul(ssp3[:], ones_col[:], sq3[:],
                                                     start=(o == 0),
                                                     stop=(o == HCH - 1))
                                # bc3' = 8/sqrt(ms+eps) = 1/sqrt(ss8/HID + eps/64)
                                bc3 = inv_chain(ssp3, TW, 1.0 / HID, eps64[:],
                                                f"b3_{t}")
                                for o in range(HCH):
                                    nc.vector.tensor_tensor(h2[:, o, :], h2[:, o, :],
                                                            bc3, MUL)

                            def attn_chunk(b, qt, fold=False):
                                tt = b * QT + qt
                                qc0 = b * S + qt * TW
                                nkt = 4 * qt + 4
                                dnp = [ps.tile([1, TW], f32, tag="r", bufs=2,
                                               name=f"dnp{h}") for h in range(HPC)]
                                atp = [ps.tile([P, TW], f32, tag="att", bufs=2,
                                               name=f"atp{h}") for h in range(HPC)]
                                exs = [[None] * nkt for _ in range(HPC)]

                                def consume(h, kt):
                                    nc.tensor.matmul(dnp[h][:], ones_col[:],
                                                     exs[h][kt][:],
                                                     start=(kt == 0),
                                                     stop=(kt == nkt - 1))
                                    nc.tensor.matmul(atp[h][:],
                                                     vnat[:, b * KT + kt,
                                                          h * DV:(h + 1) * DV],
                                                     exs[h][kt][:],
                                                     start=(kt == 0),
                                                     stop=(kt == nkt - 1))

                                # both heads interleaved: 4 independent tiles in
                                # flight keep the scores->mask->exp->consume chain
                                # off the PE critical path
                                for kt in range(nkt):
                                    kc0 = b * S + kt * P
                                    j = kt - 4 * qt
                                    for h in range(HPC):
                                        scp = ps.tile([P, TW], f32, tag="big",
                                                      bufs=4, name="scp")
                                        nc.tensor.matmul(scp[:],
                                                         knope[:, h, kc0:kc0 + P],
                                                         qsb[:, h, qc0:qc0 + TW],
                                                         start=True, stop=False)
                                        nc.tensor.matmul(
                                            scp[:],
                                            krope[h * DR:(h + 1) * DR, kc0:kc0 + P],
                                            qrope[h * DR:(h + 1) * DR, qc0:qc0 + TW],
                                            start=False, stop=True)
                                        ex = wrk.tile([P, TW], bf16, tag="ex",
                                                      bufs=6, name="ex")
                                        if j >= 0:
                                            mtmp = wrk.tile([P, TW], f32, tag="mt",
                                                            bufs=2, name="mtmp")
                                            m0 = 384 - j * P
                                            nc.vector.tensor_tensor(
                                                mtmp[:], scp[:],
                                                msk[:, m0:m0 + TW], ADD)
                                            nc.scalar.activation(ex[:], mtmp[:],
                                                                 AF.Exp)
                                        else:
                                            nc.scalar.activation(ex[:], scp[:],
                                                                 AF.Exp)
                                        exs[h][kt] = ex
                                    if kt >= 2:
                                        for h in range(HPC):
                                            consume(h, kt - 2)
                                for h in range(HPC):
                                    consume(h, max(nkt - 2, 0))
                                for h in range(HPC):
                                    if nkt > 1:
                                        consume(h, nkt - 1)
                                # 1/denom: narrow copy -> PE bcast -> wide recip
                                atns = []
                                dbcs = []
                                for h in range(HPC):
                                    drow = row.tile([1, TW], bf16, tag="nrow",
                                                    bufs=1, name="drow")
                                    nc.scalar.copy(drow[:], dnp[h][:])
                                    dbp = ps.tile([P, TW], f32, tag="big", bufs=4,
                                                  name="dbp")
                                    nc.tensor.matmul(dbp[:], ones_row[:], drow[:],
                                                     start=True, stop=True)
                                    dbc = wrk.tile([P, TW], f32, tag="inv", bufs=2,
                                                   name="dbc")
                                    nc.vector.reciprocal(dbc[:], dbp[:])
                                    dbcs.append(dbc)
                                for h in range(HPC):
                                    atn = att.tile([P, TW], bf16, tag="atn", bufs=2,
                                                   name="atn")
                                    nc.vector.tensor_tensor(atn[:], atp[h][:],
                                                            dbcs[h][:], MUL)
                                    atns.append(atn)
                                atn0, atn = atns
                                # o_proj partial (wo/8 folded) + hidden/64 -> x/8
                                for ho in range(HCH):
                                    op = ps.tile([P, TW], f32, tag="big", bufs=4,
                                                 name="op")
                                    nc.tensor.matmul(op[:],
                                                     wo_sb[:, 0, ho * P:(ho + 1) * P],
                                                     atn0[:], start=True, stop=False)
                                    nc.tensor.matmul(op[:],
                                                     wo_sb[:, 1, ho * P:(ho + 1) * P],
                                                     atn[:], start=False, stop=True)
                                    osb = wrk.tile([P, TW], bf16, tag="ex", bufs=6,
                                                   name="osb")
                                    if fold:
                                        h64 = wrk.tile([P, TW], bf16, tag="h64",
                                                       bufs=3, name="h64")
                                        nc.sync.dma_start(
                                            h64[:],
                                            h64c3.ap()[:, ho * TW:(ho + 1) * TW])
                                        nc.vector.tensor_tensor(osb[:], op[:],
                                                                h64[:], ADD)
                                    else:
                                        nc.vector.tensor_copy(out=osb[:], in_=op[:])
                                    nc.sync.dma_start(
                                        ar_in[tt][ho * P:(ho + 1) * P, :], osb[:])
                                nc.gpsimd.collective_compute(
                                    "AllReduce", ADD, ins=[ar_in[tt][:].opt()],
                                    outs=[ar_out[tt][:].opt()], replica_groups=rg)

                            def b_gateup(t):
                                h2 = h2s[t]
                                act = bp.tile([P, FPC // P, TW], bf16, tag="act",
                                              bufs=1, name=f"act_{t}")
                                for fi in range(FPC // P):
                                    gp = ps.tile([P, TW], f32, tag="big", bufs=4,
                                                 name="gp")
                                    for o in range(HCH):
                                        nc.tensor.matmul(
                                            gp[:], wg_sb[:, o, fi * P:(fi + 1) * P],
                                            h2[:, o, :],
                                            start=(o == 0), stop=(o == HCH - 1))
                                    up = ps.tile([P, TW], f32, tag="att", bufs=2,
                                                 name="up")
                                    for o in range(HCH):
                                        nc.tensor.matmul(
                                            up[:], wu_sb[:, o, fi * P:(fi + 1) * P],
                                            h2[:, o, :],
                                            start=(o == 0), stop=(o == HCH - 1))
                                    gs = wrk.tile([P, TW], bf16, tag="gs", bufs=1,
                                                  name="gs")
                                    nc.scalar.activation(gs[:], gp[:], AF.Silu)
                                    nc.vector.tensor_tensor(act[:, fi, :], up[:],
                                                            gs[:], MUL)
                                acts[t] = act

                            def b_down(t, folded=False):
                                act = acts[t]
                                for ho in range(HCH):
                                    dpp = ps.tile([P, TW], f32, tag="big",
                                                  bufs=4, name="dpp")
                                    for c in range(FPC // P):
                                        nc.tensor.matmul(
                                            dpp[:],
                                            wd_sb[:, c, ho * P:(ho + 1) * P],
                                            act[:, c, :],
                                            start=(c == 0),
                                            stop=(c == FPC // P - 1))
                                    xo = wrk.tile([P, TW], bf16, tag="h64",
                                                  bufs=3, name="xo")
                                    nc.sync.dma_start(
                                        xo[:], ar_out[t][ho * P:(ho + 1) * P, :])
                                    dsb = wrk.tile([P, TW], bf16, tag="ex",
                                                   bufs=6, name="dsb")
                                    if folded:
                                        nc.vector.tensor_tensor(dsb[:], dpp[:],
                                                                xo[:], ADD)
                                    else:
                                        xh = wrk.tile([P, TW], bf16, tag="h64",
                                                      bufs=3, name="xh")
                                        nc.sync.dma_start(
                                            xh[:],
                                            htb8.ap()[t, :, ho * TW:(ho + 1) * TW])
                                        nc.vector.tensor_tensor(dsb[:], dpp[:],
                                                                xo[:], ADD)
                                        nc.vector.tensor_tensor(dsb[:], dsb[:],
                                                                xh[:], ADD)
                                    nc.sync.dma_start(
                                        rs_in[t][ho * P:(ho + 1) * P, :], dsb[:])
                                nc.gpsimd.collective_compute(
                                    "ReduceScatter", ADD, ins=[rs_in[t][:].opt()],
                                    outs=[rs_out[t][:].opt()], replica_groups=rg)
                                nc.sync.dma_start(out.ap()[t], rs_out[t][:])

                            attn_chunk(0, 0)
                            b_pre(0)
                            attn_chunk(0, 1)
                            b_pre(1)
                            attn_chunk(1, 0)
                            attn_chunk(1, 1, fold=True)
                            b_fin(0)
                            b_gateup(0)
                            b_fin(1)
                            b_pre(2)
                            b_down(0)
                            b_gateup(1)
                            b_fin(2)
                            b_pre3()
                            b_down(1)
                            b_gateup(2)
                            b_fin(3, folded=True)
                            b_down(2)
                            b_gateup(3)
                            b_down(3, folded=True)
    nc.compile()
    return nc


def _row_major(w, blocks, width):
    # [blocks*P, width] -> [P, blocks*width] partition rows
    return np.ascontiguousarray(
        w.reshape(blocks, P, width).transpose(1, 0, 2).reshape(P, blocks * width))


def _prep(hidden_states, positions, w_in_ln, w_q, w_kv_a, w_kv_a_ln,
          w_kv_b, w_o, w_post_ln, w_gate, w_up, w_down):
    hT = np.ascontiguousarray(
        np.asarray(hidden_states, np.float32).reshape(T, HID).T)

    pos = np.asarray(positions).reshape(-1).astype(np.float64)
    inv = ROPE_BASE ** (-np.arange(0, DR, 2, dtype=np.float64) / DR)
    fr = pos[:, None] * inv[None, :]                      # [T, 32]
    c32 = np.cos(fr).T.astype(np.float32)                 # [32, T]
    s32 = np.sin(fr).T.astype(np.float32)
    cosf = np.concatenate([c32, c32, c32, c32], 0)        # [128, T], dup halves
    sinf = np.concatenate([-s32, s32, -s32, s32], 0)

    r = np.arange(P)[:, None]
    u = np.arange(TW + 384)[None, :]
    masks = np.where(u >= r + 384, 0.0, NEG).astype(np.float32)   # [128, 896]

    w_in_ln = np.asarray(w_in_ln, np.float32)
    wqf = (np.asarray(w_q, np.float32) * w_in_ln[:, None] * SCALING
           ).reshape(HID, H, DQK)
    wkvaf = np.asarray(w_kv_a, np.float32) * w_in_ln[:, None]
    kpe_w = wkvaf[:, KVR:]
    wkva_p = np.concatenate([wkvaf[:, :KVR], kpe_w[:, 0::2], kpe_w[:, 1::2]], 1)
    wkvbf = (np.asarray(w_kv_b, np.float32)
             * np.asarray(w_kv_a_ln, np.float32)[:, None]).reshape(KVR, H, DN + DV)
    w_post_ln = np.asarray(w_post_ln, np.float32)
    wgf = np.asarray(w_gate, np.float32) * w_post_ln[:, None]
    wuf = np.asarray(w_up, np.float32) * w_post_ln[:, None]
    wdf = np.asarray(w_down, np.float32)
    wof = (np.asarray(w_o, np.float32) / NC_N).reshape(H, DV, HID)

    htb = hT.astype(BF)
    # chunk-major partition rows: [t, p, o*TW+w] = hT[o*128+p, t*TW+w]
    def chunk_major(a):
        return np.ascontiguousarray(
            a.reshape(HCH, P, TT, TW).transpose(2, 1, 0, 3).reshape(TT, P, HCH * TW))
    htb4 = chunk_major(htb)
    htb8 = chunk_major((hT / 8.0).astype(BF))
    h64c3 = np.ascontiguousarray(chunk_major((hT / 64.0).astype(BF))[TT - 1])

    in_maps = []
    for core in range(NC_N):
        hs = [2 * core, 2 * core + 1]
        nopes = np.concatenate([wqf[:, h, :DN] for h in hs], 1)
        pes = []
        for h in hs:
            pe = wqf[:, h, DN:]
            pes += [pe[:, 0::2], pe[:, 1::2]]
        wq_c = np.concatenate([nopes] + pes, 1)
        c0 = core * SH
        in_maps.append({
            "htb": htb4,
            "htb8": htb8,
            "h64c3": h64c3,
            "htsh": _row_major(np.ascontiguousarray(htb[:, c0:c0 + SH]).astype(
                np.float32), HCH, SH).astype(BF),
            "wq": _row_major(wq_c, HCH, HPC * DQK).astype(BF),
            "wkva": np.ascontiguousarray(
                np.pad(wkva_p, ((0, 0), (0, (KVC + 1) * P - (KVR + DR))))
                .reshape(HCH, P, KVC + 1, P).transpose(1, 2, 0, 3)
                .reshape(P, (KVC + 1) * HCH * P)).astype(BF),
            "wkvbn": _row_major(
                np.concatenate([wkvbf[:, h, :DN] for h in hs], 1),
                KVC, HPC * DN).astype(BF),
            "wkvbv": _row_major(
                np.concatenate([wkvbf[:, h, DN:] for h in hs], 1),
                KVC, HPC * DV).astype(BF),
            "wo": _row_major(np.concatenate([wof[h] for h in hs], 0),
                             HPC, HID).astype(BF),
            "wg": _row_major(wgf[:, core * FPC:(core + 1) * FPC],
                             HCH, FPC).astype(BF),
            "wu": _row_major(wuf[:, core * FPC:(core + 1) * FPC],
                             HCH, FPC).astype(BF),
            "wd": _row_major(wdf[core * FPC:(core + 1) * FPC, :],
                             FPC // P, HID).astype(BF),
            "cosf": cosf.astype(BF),
            "sinf": sinf.astype(BF),
            "cossh": cosf[0:DR, c0:c0 + SH].astype(BF).copy(),
            "sinsh": sinf[0:DR, c0:c0 + SH].astype(BF).copy(),
            "masks": masks.astype(BF),
        })
    return in_maps


def kernel(**inputs):
    if "nc" not in _CACHE:
        _CACHE["nc"] = _build()
    nc = _CACHE["nc"]
    in_maps = _prep(**inputs)
    res = run_bass_kernel_spmd(nc, in_maps, core_ids=list(range(NC_N)))
    # o: per-core [TT, HID//NC_N, TW] bf16 -> full [HID, T] f32
    outT = np.concatenate(
        [np.concatenate(list(res.results[c]["o"].astype(np.float32)), axis=1)
         for c in range(NC_N)], 0)
    return np.ascontiguousarray(outT.T).reshape(B, S, HID).astype(np.float32)


# revision 36
# speedup vs baseline: 1.0552x; 1.0552x over previous
"""DeepseekV2 decoder layer on 8 TRN2 NeuronCores (Bass/Tile).

Sharding: TP over heads (2/core) for q/kv_b/attention/o_proj, kv_a
token-sharded (256 tokens/core) + AllGather, TP over INTER (1024/core) for
the MLP. Chunked AllReduce after o_proj (carrying x/8 = (hidden+attn)/8,
with wo/8 and hidden/64 folded in) and chunked ReduceScatter after
down_proj.

Internal layout is feature-major ("transposed"): activations live as
[feature, token] so every matmul output feeds the next as `rhs` without any
on-device transpose. All large DRAM inputs are laid out host-side as
[128, free] partition-rows so every SBUF load is one DMA with >=2KB
contiguous runs. ht streams token-chunk-major with norm+q-proj interleaved
per chunk; MLP weights prefetch behind the attention inputs; B-chunk norm
preps run inside the attention window in the space freed by the kv_b pool.
"""

import numpy as np
import ml_dtypes

import concourse.bass as bass
import concourse.mybir as mybir
import concourse.tile as tile
from concourse import bacc
from concourse.bass_utils import run_bass_kernel_spmd

BF = ml_dtypes.bfloat16

B, S, HID = 2, 1024, 2048
T = B * S                      # 2048 tokens
H = 16
DN, DR = 128, 64
DQK = DN + DR
DV = 128
KVR = 512
INTER = 8192
EPS = 1e-6
ROPE_BASE = 10000.0
SCALING = DQK ** -0.5

NC_N = 8
HPC = H // NC_N                # 2 heads per core
FPC = INTER // NC_N            # 1024 inter per core
P = 128
HCH = HID // P                 # 16 hid chunks
TT = 4                         # token chunks of 512
TW = T // TT                   # 512
SH = T // NC_N                 # 256-token kv_a shard per core
KT = S // P                    # 8 k-tiles of 128 per batch
QT = S // TW                   # 2 q-chunks of 512 per batch
KVC = KVR // P                 # 4
AGW = KVC * SH + SH            # 1280: kva (f-major) + kpe corner block
NEG = -30000.0

f32 = mybir.dt.float32
bf16 = mybir.dt.bfloat16
ADD = mybir.AluOpType.add
MUL = mybir.AluOpType.mult
AF = mybir.ActivationFunctionType

_CACHE = {}


def _build():
    nc = bacc.Bacc("TRN2", target_bir_lowering=False, debug=False, num_devices=NC_N)
    dp = lambda n, sh, dt: nc.dram_tensor(n, sh, dt, kind="ExternalInput")
    htb = dp("htb", [TT, P, HCH * TW], bf16)        # chunk-major partition rows
    htb8 = dp("htb8", [TT, P, HCH * TW], bf16)      # hidden/8, chunk-major rows
    h64c3 = dp("h64c3", [P, HCH * TW], bf16)        # hidden/64, chunk 3 only
    htsh = dp("htsh", [P, HCH * SH], bf16)          # this core's kv_a token shard
    wq = dp("wq", [P, HCH * HPC * DQK], bf16)
    wkva = dp("wkva", [P, (KVC + 1) * HCH * P], bf16)   # f-major blocks
    wkvbn = dp("wkvbn", [P, KVC * HPC * DN], bf16)
    wkvbv = dp("wkvbv", [P, KVC * HPC * DV], bf16)
    wo = dp("wo", [P, HPC * HID], bf16)             # pre-divided by 8
    wg = dp("wg", [P, HCH * FPC], bf16)
    wu = dp("wu", [P, HCH * FPC], bf16)
    wd = dp("wd", [P, (FPC // P) * HID], bf16)
    cosf = dp("cosf", [P, T], bf16)      # rows 64:128 duplicate 0:64
    sinf = dp("sinf", [P, T], bf16)
    cossh = dp("cossh", [DR, SH], bf16)
    sinsh = dp("sinsh", [DR, SH], bf16)
    masks = dp("masks", [P, TW + 384], bf16)        # shifted-window causal mask
    out = nc.dram_tensor("o", [TT, HID // NC_N, TW], bf16, kind="ExternalOutput")
    rg = [list(range(NC_N))]

    with tile.TileContext(nc) as tc:
        with tc.tile_pool(name="const", bufs=1) as cpool, \
             tc.tile_pool(name="dram", bufs=1, space="DRAM") as dram, \
             tc.tile_pool(name="ps", bufs=1, space="PSUM") as ps, \
             tc.tile_pool(name="wrk", bufs=3) as wrk, \
             tc.tile_pool(name="row", bufs=2) as row:
            ones_col = cpool.tile([P, 1], bf16)
            nc.vector.memset(ones_col[:], 1.0)
            ones_row = cpool.tile([1, P], bf16)
            nc.vector.memset(ones_row[:], 1.0)
            epsb = cpool.tile([P, 1], f32)
            nc.vector.memset(epsb[:], EPS)
            eps64 = cpool.tile([P, 1], f32)
            nc.vector.memset(eps64[:], EPS / 64.0)

            ag_in = dram.tile([P, AGW], bf16, name="ag_in")
            ag_out = dram.tile([NC_N, P, AGW], bf16, addr_space="Shared",
                               name="ag_out")
            ar_in = [dram.tile([HID, TW], bf16, name=f"ar_in{t}") for t in range(TT)]
            ar_out = [dram.tile([HID, TW], bf16, addr_space="Shared",
                                name=f"ar_out{t}") for t in range(TT)]
            rs_in = [dram.tile([HID, TW], bf16, name=f"rs_in{t}")
                     for t in range(TT)]
            rs_out = [dram.tile([HID // NC_N, TW], bf16, name=f"rs_out{t}")
                      for t in range(TT)]

            # helper: [1,W] f32 PSUM sumsq row -> wide f32 inv-scale [P,W]
            def inv_chain(ssp, w, scale, bias, name, out=None):
                nrow = row.tile([1, TW], bf16, tag="nrow", bufs=1, name=f"nr_{name}")
                nc.scalar.copy(nrow[:, :w], ssp[:, :w])
                bcp = ps.tile([P, TW], f32, tag="big", bufs=4, name=f"bc_{name}")
                nc.tensor.matmul(bcp[:, :w], ones_row[:], nrow[:, :w],
                                 start=True, stop=True)
                sd = wrk.tile([P, TW], f32, tag="sd", bufs=1, name=f"sd_{name}")
                nc.scalar.activation(sd[:, :w], bcp[:, :w], AF.Sqrt,
                                     bias=bias, scale=scale)
                if out is None:
                    out = wrk.tile([P, TW], f32, tag="inv", bufs=2,
                                   name=f"inv_{name}")[:, :w]
                nc.vector.reciprocal(out, sd[:, :w])
                return out

            with tc.tile_pool(name="pers", bufs=1) as pers:
                qsb = pers.tile([P, 3, T], bf16)           # 12K
                qrope = pers.tile([P, T], bf16)            # head h at rows h*64
                krope = pers.tile([P, T], bf16)            # both halves identical

                # ---- A0: kv_a shard + AllGather (fires early) ----
                with tc.tile_pool(name="shp", bufs=1) as shp:
                    hts = shp.tile([P, HCH, SH], bf16)
                    nc.sync.dma_start(hts[:], htsh.ap())
                    wkva_sb = shp.tile([P, KVC + 1, HCH, P], bf16)   # 20K
                    for f in range(KVC + 1):
                        nc.sync.dma_start(wkva_sb[:, f, :, :],
                                          wkva.ap()[:, f * HCH * P:(f + 1) * HCH * P])
                    css = shp.tile([DR, SH], bf16)
                    nc.sync.dma_start(css[:], cossh.ap())
                    sns = shp.tile([DR, SH], bf16)
                    nc.sync.dma_start(sns[:], sinsh.ap())

                    # shard input-norm scale r1_sh
                    sshp = ps.tile([1, TW], f32, tag="r", bufs=2, name="sshp")
                    for o in range(HCH):
                        sqs = wrk.tile([P, TW], bf16, tag="sq", bufs=2, name="sqs")
                        nc.scalar.square(sqs[:, :SH], hts[:, o, :])
                        nc.tensor.matmul(sshp[:, :SH], ones_col[:], sqs[:, :SH],
                                         start=(o == 0), stop=(o == HCH - 1))
                    rsh = inv_chain(sshp, SH, 1.0 / HID, epsb[:], "rsh")

                    # latent projection for the shard
                    lats = shp.tile([P, KVC * SH], bf16)
                    kpes = shp.tile([DR, SH], bf16)
                    ss2p = ps.tile([1, TW], f32, tag="r", bufs=2, name="ss2p")
                    for f in range(KVC + 1):
                        wid = P if f < KVC else DR
                        lp = ps.tile([P, TW], f32, tag="big", bufs=4, name="lp")
                        for o in range(HCH):
                            nc.tensor.matmul(lp[:wid, :SH],
                                             wkva_sb[:, f, o, :wid],
                                             hts[:, o, :],
                                             start=(o == 0), stop=(o == HCH - 1))
                        if f < KVC:
                            nc.vector.tensor_copy(out=lats[:, f * SH:(f + 1) * SH],
                                                  in_=lp[:, :SH])
                            sq2 = wrk.tile([P, TW], bf16, tag="sq", bufs=2, name="sq2")
                            nc.scalar.square(sq2[:, :SH], lp[:, :SH])
                            nc.tensor.matmul(ss2p[:, :SH], ones_col[:], sq2[:, :SH],
                                             start=(f == 0), stop=(f == KVC - 1))
                        else:
                            nc.vector.tensor_tensor(kpes[:], lp[:DR, :SH],
                                                    rsh[:DR, :SH], MUL)
                    r2sh = inv_chain(ss2p, SH, 1.0 / KVR, epsb[:], "r2sh")
                    kvas = shp.tile([P, KVC * SH], bf16)
                    for f in range(KVC):
                        nc.vector.tensor_tensor(kvas[:, f * SH:(f + 1) * SH],
                                                lats[:, f * SH:(f + 1) * SH],
                                                r2sh[:, :SH], MUL)
                    # rope k_pe shard: [x1(32); x2(32)] layout
                    ksw = wrk.tile([DR, SH], bf16, tag="rps", bufs=1, name="ksw")
                    nc.sync.dma_start(ksw[0:32, :], kpes[32:64, :])
                    nc.sync.dma_start(ksw[32:64, :], kpes[0:32, :])
                    ktmp = wrk.tile([DR, SH], bf16, tag="rps", bufs=1, name="ktmp")
                    nc.vector.tensor_tensor(ktmp[:], kpes[:], css[:], MUL)
                    krs = shp.tile([DR, SH], bf16)
                    nc.vector.tensor_tensor(krs[:], ksw[:], sns[:], MUL)
                    nc.vector.tensor_tensor(krs[:], krs[:], ktmp[:], ADD)
                    # pack shard -> ag_in and AllGather
                    nc.sync.dma_start(ag_in[:, 0:KVC * SH], kvas[:])
                    nc.sync.dma_start(ag_in[0:DR, KVC * SH:KVC * SH + SH], krs[:])
                    nc.gpsimd.collective_compute(
                        "AllGather", mybir.AluOpType.bypass, ins=[ag_in[:].opt()],
                        outs=[ag_out[:].opt()], replica_groups=rg)

                # ---- A1: full input norm + q projection + q rope ----
                with tc.tile_pool(name="a1", bufs=1) as a1:
                    wq_sb = a1.tile([P, HCH, HPC * DQK], bf16)   # 12K
                    nc.sync.dma_start(wq_sb[:], wq.ap())
                    ht = a1.tile([P, TT, HCH, TW], bf16)         # 64K
                    for t in range(TT):
                        for g in range(4):
                            nc.sync.dma_start(
                                ht[:, t, 4 * g:4 * g + 4, :],
                                htb.ap()[t, :, 4 * g * TW:(4 * g + 4) * TW])
                    cs = a1.tile([P, T], bf16)
                    nc.sync.dma_start(cs[:], cosf.ap())
                    sn = a1.tile([P, T], bf16)
                    nc.sync.dma_start(sn[:], sinf.ap())
                    bc1 = a1.tile([P, TT, TW], f32)              # 8K

                    for t in range(TT):
                        ssp = ps.tile([1, TW], f32, tag="r", bufs=2, name="ssp")
                        for o in range(HCH):
                            sq = wrk.tile([P, TW], bf16, tag="sq", bufs=2, name="sq")
                            nc.scalar.square(sq[:], ht[:, t, o, :])
                            nc.tensor.matmul(ssp[:], ones_col[:], sq[:],
                                             start=(o == 0), stop=(o == HCH - 1))
                        inv_chain(ssp, TW, 1.0 / HID, epsb[:], f"r1_{t}",
                                  out=bc1[:, t, :])
                        # q projection for this chunk (SCALING folded into wq)
                        for f in range(3):
                            qp = ps.tile([P, TW], f32, tag="big", bufs=4, name="qp")
                            for o in range(HCH):
                                nc.tensor.matmul(qp[:], wq_sb[:, o, f * P:(f + 1) * P],
                                                 ht[:, t, o, :],
                                                 start=(o == 0), stop=(o == HCH - 1))
                            nc.vector.tensor_tensor(qsb[:, f, t * TW:(t + 1) * TW],
                                                    qp[:], bc1[:, t, :], MUL)

                    # q rope: [x1(32); x2(32)] per head, head h on rows h*64
                    src = qsb[:, 2, :]
                    sw = a1.tile([P, T], bf16, tag="rope", bufs=2, name="qsw")
                    for h in range(HPC):
                        nc.sync.dma_start(sw[h * DR:h * DR + 32, :],
                                          src[h * DR + 32:h * DR + 64, :])
                        nc.sync.dma_start(sw[h * DR + 32:h * DR + 64, :],
                                          src[h * DR:h * DR + 32, :])
                    tmp = a1.tile([P, T], bf16, tag="rope", bufs=2, name="qtmp")
                    nc.vector.tensor_tensor(tmp[:], src, cs[:], MUL)
                    nc.vector.tensor_tensor(qrope[:], sw[:], sn[:], MUL)
                    nc.vector.tensor_tensor(qrope[:], qrope[:], tmp[:], ADD)
                    # k rope full comes pre-roped from the AllGather (both halves)
                    for c in range(NC_N):
                        for h in range(HPC):
                            nc.sync.dma_start(
                                krope[h * DR:(h + 1) * DR, c * SH:(c + 1) * SH],
                                ag_out[c, 0:DR, KVC * SH:KVC * SH + SH])

                # ---- bw: MLP weights (DMAs issued after attention inputs) ----
                with tc.tile_pool(name="bw", bufs=1) as bw:
                    wg_sb = bw.tile([P, HCH, FPC], bf16)       # 32K
                    wu_sb = bw.tile([P, HCH, FPC], bf16)       # 32K
                    wd_sb = bw.tile([P, FPC // P, HID], bf16)  # 16K

                    with tc.tile_pool(name="att", bufs=1) as att:
                        knope = att.tile([P, HPC, T], bf16)        # 8K
                        vnat = att.tile([P, T // P, HPC * DV], bf16)  # 8K
                        wo_sb = att.tile([P, HPC, HID], bf16)      # 8K
                        msk = att.tile([P, TW + 384], bf16)        # 1.75K
                        nc.sync.dma_start(msk[:], masks.ap())
                        nc.sync.dma_start(wo_sb[:], wo.ap())

                        with tc.tile_pool(name="kvp", bufs=1) as kvp:
                            # kva laid out core-major so each load is contiguous
                            kva = kvp.tile([P, NC_N, KVC, SH], bf16)   # 16K
                            for c in range(NC_N):
                                nc.sync.dma_start(kva[:, c, :, :],
                                                  ag_out[c, :, 0:KVC * SH])
                            wkvbn_sb = kvp.tile([P, KVC, HPC * DN], bf16)
                            nc.sync.dma_start(wkvbn_sb[:], wkvbn.ap())
                            wkvbv_sb = kvp.tile([P, KVC, HPC * DV], bf16)
                            nc.sync.dma_start(wkvbv_sb[:], wkvbv.ap())
                            # now queue the MLP weight prefetch behind these
                            nc.sync.dma_start(wg_sb[:], wg.ap())
                            nc.sync.dma_start(wu_sb[:], wu.ap())
                            nc.sync.dma_start(wd_sb[:], wd.ap())

                            for h in range(HPC):
                                for t in range(TT):
                                    kp = ps.tile([P, TW], f32, tag="big", bufs=4,
                                                 name="kp")
                                    for c in range(KVC):
                                        nc.tensor.matmul(
                                            kp[:], wkvbn_sb[:, c, h * P:(h + 1) * P],
                                            kva[:, 2 * t:2 * t + 2, c, :],
                                            start=(c == 0), stop=(c == KVC - 1))
                                    nc.vector.tensor_copy(
                                        out=knope[:, h, t * TW:(t + 1) * TW], in_=kp[:])
                            for to in range(T // P):
                                vp = ps.tile([P, TW], f32, tag="att", bufs=2,
                                             name="vp")
                                co, po = to // 2, (to % 2) * P
                                for c in range(KVC):
                                    nc.tensor.matmul(
                                        vp[:, :HPC * DV],
                                        kva[:, co, c, po:po + P],
                                        wkvbv_sb[:, c, :],
                                        start=(c == 0), stop=(c == KVC - 1))
                                nc.vector.tensor_copy(out=vnat[:, to, :],
                                                      in_=vp[:, :HPC * DV])

                        # B-chunk tiles live in the space kvp just freed
                        with tc.tile_pool(name="bp", bufs=1) as bp:
                            h2s = [None] * TT
                            acts = [None] * TT

                            def b_pre(t):
                                # x/8 = hidden/8 + ar_out[t] (attn/8); h2 doubles
                                # as the x/8 store. Fat chunk DMA, no AR dep.
                                h2 = bp.tile([P, HCH, TW], bf16, tag="h2", bufs=2,
                                             name=f"h2_{t}")
                                for g in range(4):
                                    nc.sync.dma_start(
                                        h2[:, 4 * g:4 * g + 4, :],
                                        htb8.ap()[t, :, 4 * g * TW:(4 * g + 4) * TW])
                                h2s[t] = h2

                            def b_pre3():
                                h2s[3] = bp.tile([P, HCH, TW], bf16, tag="h2",
                                                 bufs=2, name="h2_3")

                            def b_fin(t, folded=False):
                                # folded: ar_out[t] already holds x/8
                                h2 = h2s[t]
                                ssp3 = ps.tile([1, TW], f32, tag="r", bufs=2,
                                               name="ssp3")
                                for o in range(HCH):
                                    if folded:
                                        nc.sync.dma_start(
                                            h2[:, o, :],
                                            ar_out[t][o * P:(o + 1) * P, :])
                                    else:
                                        aro = wrk.tile([P, TW], bf16, tag="h64",
                                                       bufs=3, name="aro")
                                        nc.sync.dma_start(
                                            aro[:], ar_out[t][o * P:(o + 1) * P, :])
                                        nc.vector.tensor_tensor(h2[:, o, :],
                                                                h2[:, o, :],
                                                                aro[:], ADD)
                                    sq3 = wrk.tile([P, TW], bf16, tag="sq", bufs=2,
                                                   name="sq3")
                                    nc.scalar.square(sq3[:], h2[:, o, :])
                                    nc.tensor.matmul(ssp3[:], ones_col[:], sq3[:],
                                                     start=(o == 0),
                                                     stop=(o == HCH - 1))
                                # bc3' = 8/sqrt(ms+eps) = 1/sqrt(ss8/HID + eps/64)
                                bc3 = inv_chain(ssp3, TW, 1.0 / HID, eps64[:],
                                                f"b3_{t}")
                                for o in range(HCH):
                                    nc.vector.tensor_tensor(h2[:, o, :], h2[:, o, :],
                                                            bc3, MUL)

                            def attn_chunk(b, qt, fold=False):
                                tt = b * QT + qt
                                qc0 = b * S + qt * TW
                                nkt = 4 * qt + 4
                                dnp = [ps.tile([1, TW], f32, tag="r", bufs=2,
                                               name=f"dnp{h}") for h in range(HPC)]
                                atp = [ps.tile([P, TW], f32, tag="att", bufs=2,
                                               name=f"atp{h}") for h in range(HPC)]
                                exs = [[None] * nkt for _ in range(HPC)]

                                def consume(h, kt):
                                    nc.tensor.matmul(dnp[h][:], ones_col[:],
                                                     exs[h][kt][:],
                                                     start=(kt == 0),
                                                     stop=(kt == nkt - 1))
                                    nc.tensor.matmul(atp[h][:],
                                                     vnat[:, b * KT + kt,
                                                          h * DV:(h + 1) * DV],
                                                     exs[h][kt][:],
                                                     start=(kt == 0),
                                                     stop=(kt == nkt - 1))

                                # both heads interleaved: 4 independent tiles in
                                # flight keep the scores->mask->exp->consume chain
                                # off the PE critical path
                                for kt in range(nkt):
                                    kc0 = b * S + kt * P
                                    j = kt - 4 * qt
                                    for h in range(HPC):
                                        scp = ps.tile([P, TW], f32, tag="big",
                                                      bufs=4, name="scp")
                                        nc.tensor.matmul(scp[:],
                                                         knope[:, h, kc0:kc0 + P],
                                                         qsb[:, h, qc0:qc0 + TW],
                                                         start=True, stop=False)
                                        nc.tensor.matmul(
                                            scp[:],
                                            krope[h * DR:(h + 1) * DR, kc0:kc0 + P],
                                            qrope[h * DR:(h + 1) * DR, qc0:qc0 + TW],
                                            start=False, stop=True)
                                        ex = wrk.tile([P, TW], bf16, tag="ex",
                                                      bufs=6, name="ex")
                                        if j >= 0:
                                            mtmp = wrk.tile([P, TW], f32, tag="mt",
                                                            bufs=2, name="mtmp")
                                            m0 = 384 - j * P
                                            nc.vector.tensor_tensor(
                                                mtmp[:], scp[:],
                                                msk[:, m0:m0 + TW], ADD)
                                            nc.scalar.activation(ex[:], mtmp[:],
                                                                 AF.Exp)
                                        else:
                                            nc.scalar.activation(ex[:], scp[:],
                                                                 AF.Exp)
                                        exs[h][kt] = ex
                                    if kt >= 2:
                                        for h in range(HPC):
                                            consume(h, kt - 2)
                                for h in range(HPC):
                                    consume(h, max(nkt - 2, 0))
                                for h in range(HPC):
                                    if nkt > 1:
                                        consume(h, nkt - 1)
                                # 1/denom: narrow copy -> PE bcast -> wide recip
                                atns = []
                                dbcs = []
                                for h in range(HPC):
                                    drow = row.tile([1, TW], bf16, tag="nrow",
                                                    bufs=1, name="drow")
                                    nc.scalar.copy(drow[:], dnp[h][:])
                                    dbp = ps.tile([P, TW], f32, tag="big", bufs=4,
                                                  name="dbp")
                                    nc.tensor.matmul(dbp[:], ones_row[:], drow[:],
                                                     start=True, stop=True)
                                    dbc = wrk.tile([P, TW], f32, tag="inv", bufs=2,
                                                   name="dbc")
                                    nc.vector.reciprocal(dbc[:], dbp[:])
                                    dbcs.append(dbc)
                                for h in range(HPC):
                                    atn = att.tile([P, TW], bf16, tag="atn", bufs=2,
                                                   name="atn")
                                    nc.vector.tensor_tensor(atn[:], atp[h][:],
                                                            dbcs[h][:], MUL)
                                    atns.append(atn)
                                atn0, atn = atns
                                # o_proj partial (wo/8 folded) + hidden/64 -> x/8
                                for ho in range(HCH):
                                    op = ps.tile([P, TW], f32, tag="big", bufs=4,
                                                 name="op")
                                    nc.tensor.matmul(op[:],
                                                     wo_sb[:, 0, ho * P:(ho + 1) * P],
                                                     atn0[:], start=True, stop=False)
                                    nc.tensor.matmul(op[:],
                                                     wo_sb[:, 1, ho * P:(ho + 1) * P],
                                                     atn[:], start=False, stop=True)
                                    osb = wrk.tile([P, TW], bf16, tag="ex", bufs=6,
                                                   name="osb")
                                    if fold:
                                        h64 = wrk.tile([P, TW], bf16, tag="h64",
                                                       bufs=3, name="h64")
                                        nc.sync.dma_start(
                                            h64[:],
                                            h64c3.ap()[:, ho * TW:(ho + 1) * TW])
                                        nc.vector.tensor_tensor(osb[:], op[:],
                                                                h64[:], ADD)
                                    else:
                                        nc.vector.tensor_copy(out=osb[:], in_=op[:])
                                    nc.sync.dma_start(
                                        ar_in[tt][ho * P:(ho + 1) * P, :], osb[:])
                                nc.gpsimd.collective_compute(
                                    "AllReduce", ADD, ins=[ar_in[tt][:].opt()],
                                    outs=[ar_out[tt][:].opt()], replica_groups=rg)

                            def b_gateup(t):
                                h2 = h2s[t]
                                act = bp.tile([P, FPC // P, TW], bf16, tag="act",
                                              bufs=1, name=f"act_{t}")
                                for fi in range(FPC // P):
                                    gp = ps.tile([P, TW], f32, tag="big", bufs=4,
                                                 name="gp")
                                    for o in range(HCH):
                                        nc.tensor.matmul(
                                            gp[:], wg_sb[:, o, fi * P:(fi + 1) * P],
                                            h2[:, o, :],
                                            start=(o == 0), stop=(o == HCH - 1))
                                    up = ps.tile([P, TW], f32, tag="att", bufs=2,
                                                 name="up")
                                    for o in range(HCH):
                                        nc.tensor.matmul(
                                            up[:], wu_sb[:, o, fi * P:(fi + 1) * P],
                                            h2[:, o, :],
                                            start=(o == 0), stop=(o == HCH - 1))
                                    gs = wrk.tile([P, TW], bf16, tag="gs", bufs=1,
                                                  name="gs")
                                    nc.scalar.activation(gs[:], gp[:], AF.Silu)
                                    nc.vector.tensor_tensor(act[:, fi, :], up[:],
                                                            gs[:], MUL)
                                acts[t] = act

                            def b_down(t, folded=False):
                                act = acts[t]
                                for ho in range(HCH):
                                    dpp = ps.tile([P, TW], f32, tag="big",
                                                  bufs=4, name="dpp")
                                    for c in range(FPC // P):
                                        nc.tensor.matmul(
                                            dpp[:],
                                            wd_sb[:, c, ho * P:(ho + 1) * P],
                                            act[:, c, :],
                                            start=(c == 0),
                                            stop=(c == FPC // P - 1))
                                    xo = wrk.tile([P, TW], bf16, tag="h64",
                                                  bufs=3, name="xo")
                                    nc.sync.dma_start(
                                        xo[:], ar_out[t][ho * P:(ho + 1) * P, :])
                                    dsb = wrk.tile([P, TW], bf16, tag="ex",
                                                   bufs=6, name="dsb")
                                    if folded:
                                        nc.vector.tensor_tensor(dsb[:], dpp[:],
                                                                xo[:], ADD)
                                    else:
                                        xh = wrk.tile([P, TW], bf16, tag="h64",
                                                      bufs=3, name="xh")
                                        nc.sync.dma_start(
                                            xh[:],
                                            htb8.ap()[t, :, ho * TW:(ho + 1) * TW])
                                        nc.vector.tensor_tensor(dsb[:], dpp[:],
                                                                xo[:], ADD)
                                        nc.vector.tensor_tensor(dsb[:], dsb[:],
                                                                xh[:], ADD)
                                    nc.sync.dma_start(
                                        rs_in[t][ho * P:(ho + 1) * P, :], dsb[:])
                                nc.gpsimd.collective_compute(
                                    "ReduceScatter", ADD, ins=[rs_in[t][:].opt()],
                                    outs=[rs_out[t][:].opt()], replica_groups=rg)
                                nc.sync.dma_start(out.ap()[t], rs_out[t][:])

                            attn_chunk(0, 0)
                            b_pre(0)
                            attn_chunk(0, 1)
                            b_pre(1)
                            attn_chunk(1, 0)
                            attn_chunk(1, 1)
                            b_fin(0)
                            b_gateup(0)
                            b_fin(1)
                            b_pre(2)
                            b_down(0)
                            b_gateup(1)
                            b_fin(2)
                            b_pre(3)
                            b_down(1)
                            b_gateup(2)
                            b_fin(3)
                            b_down(2)
                            b_gateup(3)
                            b_down(3)
    nc.compile()
    return nc


def _row_major(w, blocks, width):
    # [blocks*P, width] -> [P, blocks*width] partition rows
    return np.ascontiguousarray(
        w.reshape(blocks, P, width).transpose(1, 0, 2).reshape(P, blocks * width))


def _prep(hidden_states, positions, w_in_ln, w_q, w_kv_a, w_kv_a_ln,
          w_kv_b, w_o, w_post_ln, w_gate, w_up, w_down):
    hT = np.ascontiguousarray(
        np.asarray(hidden_states, np.float32).reshape(T, HID).T)

    pos = np.asarray(positions).reshape(-1).astype(np.float64)
    inv = ROPE_BASE ** (-np.arange(0, DR, 2, dtype=np.float64) / DR)
    fr = pos[:, None] * inv[None, :]                      # [T, 32]
    c32 = np.cos(fr).T.astype(np.float32)                 # [32, T]
    s32 = np.sin(fr).T.astype(np.float32)
    cosf = np.concatenate([c32, c32, c32, c32], 0)        # [128, T], dup halves
    sinf = np.concatenate([-s32, s32, -s32, s32], 0)

    r = np.arange(P)[:, None]
    u = np.arange(TW + 384)[None, :]
    masks = np.where(u >= r + 384, 0.0, NEG).astype(np.float32)   # [128, 896]

    w_in_ln = np.asarray(w_in_ln, np.float32)
    wqf = (np.asarray(w_q, np.float32) * w_in_ln[:, None] * SCALING
           ).reshape(HID, H, DQK)
    wkvaf = np.asarray(w_kv_a, np.float32) * w_in_ln[:, None]
    kpe_w = wkvaf[:, KVR:]
    wkva_p = np.concatenate([wkvaf[:, :KVR], kpe_w[:, 0::2], kpe_w[:, 1::2]], 1)
    wkvbf = (np.asarray(w_kv_b, np.float32)
             * np.asarray(w_kv_a_ln, np.float32)[:, None]).reshape(KVR, H, DN + DV)
    w_post_ln = np.asarray(w_post_ln, np.float32)
    wgf = np.asarray(w_gate, np.float32) * w_post_ln[:, None]
    wuf = np.asarray(w_up, np.float32) * w_post_ln[:, None]
    wdf = np.asarray(w_down, np.float32)
    wof = (np.asarray(w_o, np.float32) / NC_N).reshape(H, DV, HID)

    htb = hT.astype(BF)
    # chunk-major partition rows: [t, p, o*TW+w] = hT[o*128+p, t*TW+w]
    def chunk_major(a):
        return np.ascontiguousarray(
            a.reshape(HCH, P, TT, TW).transpose(2, 1, 0, 3).reshape(TT, P, HCH * TW))
    htb4 = chunk_major(htb)
    htb8 = chunk_major((hT / 8.0).astype(BF))
    h64c3 = np.ascontiguousarray(chunk_major((hT / 64.0).astype(BF))[TT - 1])

    in_maps = []
    for core in range(NC_N):
        hs = [2 * core, 2 * core + 1]
        nopes = np.concatenate([wqf[:, h, :DN] for h in hs], 1)
        pes = []
        for h in hs:
            pe = wqf[:, h, DN:]
            pes += [pe[:, 0::2], pe[:, 1::2]]
        wq_c = np.concatenate([nopes] + pes, 1)
        c0 = core * SH
        in_maps.append({
            "htb": htb4,
            "htb8": htb8,
            "h64c3": h64c3,
            "htsh": _row_major(np.ascontiguousarray(htb[:, c0:c0 + SH]).astype(
                np.float32), HCH, SH).astype(BF),
            "wq": _row_major(wq_c, HCH, HPC * DQK).astype(BF),
            "wkva": np.ascontiguousarray(
                np.pad(wkva_p, ((0, 0), (0, (KVC + 1) * P - (KVR + DR))))
                .reshape(HCH, P, KVC + 1, P).transpose(1, 2, 0, 3)
                .reshape(P, (KVC + 1) * HCH * P)).astype(BF),
            "wkvbn": _row_major(
                np.concatenate([wkvbf[:, h, :DN] for h in hs], 1),
                KVC, HPC * DN).astype(BF),
            "wkvbv": _row_major(
                np.concatenate([wkvbf[:, h, DN:] for h in hs], 1),
                KVC, HPC * DV).astype(BF),
            "wo": _row_major(np.concatenate([wof[h] for h in hs], 0),
                             HPC, HID).astype(BF),
            "wg": _row_major(wgf[:, core * FPC:(core + 1) * FPC],
                             HCH, FPC).astype(BF),
            "wu": _row_major(wuf[:, core * FPC:(core + 1) * FPC],
                             HCH, FPC).astype(BF),
            "wd": _row_major(wdf[core * FPC:(core + 1) * FPC, :],
                             FPC // P, HID).astype(BF),
            "cosf": cosf.astype(BF),
            "sinf": sinf.astype(BF),
            "cossh": cosf[0:DR, c0:c0 + SH].astype(BF).copy(),
            "sinsh": sinf[0:DR, c0:c0 + SH].astype(BF).copy(),
            "masks": masks.astype(BF),
        })
    return in_maps


def kernel(**inputs):
    if "nc" not in _CACHE:
        _CACHE["nc"] = _build()
    nc = _CACHE["nc"]
    in_maps = _prep(**inputs)
    res = run_bass_kernel_spmd(nc, in_maps, core_ids=list(range(NC_N)))
    # o: per-core [TT, HID//NC_N, TW] bf16 -> full [HID, T] f32
    outT = np.concatenate(
        [np.concatenate(list(res.results[c]["o"].astype(np.float32)), axis=1)
         for c in range(NC_N)], 0)
    return np.ascontiguousarray(outT.T).reshape(B, S, HID).astype(np.float32)


# revision 37
# speedup vs baseline: 1.0603x; 1.0049x over previous
"""DeepseekV2 decoder layer on 8 TRN2 NeuronCores (Bass/Tile).

Sharding: TP over heads (2/core) for q/kv_b/attention/o_proj, kv_a
token-sharded (256 tokens/core) + AllGather, TP over INTER (1024/core) for
the MLP. Chunked AllReduce after o_proj (carrying x/8 = (hidden+attn)/8,
with wo/8 and hidden/64 folded in) and chunked ReduceScatter after
down_proj.

Internal layout is feature-major ("transposed"): activations live as
[feature, token] so every matmul output feeds the next as `rhs` without any
on-device transpose. All large DRAM inputs are laid out host-side as
[128, free] partition-rows so every SBUF load is one DMA with >=2KB
contiguous runs. ht streams token-chunk-major with norm+q-proj interleaved
per chunk; MLP weights prefetch behind the attention inputs; B-chunk norm
preps run inside the attention window in the space freed by the kv_b pool.
"""

import numpy as np
import ml_dtypes

import concourse.bass as bass
import concourse.mybir as mybir
import concourse.tile as tile
from concourse import bacc
from concourse.bass_utils import run_bass_kernel_spmd

BF = ml_dtypes.bfloat16

B, S, HID = 2, 1024, 2048
T = B * S                      # 2048 tokens
H = 16
DN, DR = 128, 64
DQK = DN + DR
DV = 128
KVR = 512
INTER = 8192
EPS = 1e-6
ROPE_BASE = 10000.0
SCALING = DQK ** -0.5

NC_N = 8
HPC = H // NC_N                # 2 heads per core
FPC = INTER // NC_N            # 1024 inter per core
P = 128
HCH = HID // P                 # 16 hid chunks
TT = 4                         # token chunks of 512
TW = T // TT                   # 512
SH = T // NC_N                 # 256-token kv_a shard per core
KT = S // P                    # 8 k-tiles of 128 per batch
QT = S // TW                   # 2 q-chunks of 512 per batch
KVC = KVR // P                 # 4
AGW = KVC * SH + SH            # 1280: kva (f-major) + kpe corner block
NEG = -30000.0

f32 = mybir.dt.float32
bf16 = mybir.dt.bfloat16
ADD = mybir.AluOpType.add
MUL = mybir.AluOpType.mult
AF = mybir.ActivationFunctionType

_CACHE = {}


def _build():
    nc = bacc.Bacc("TRN2", target_bir_lowering=False, debug=False, num_devices=NC_N)
    dp = lambda n, sh, dt: nc.dram_tensor(n, sh, dt, kind="ExternalInput")
    htb = dp("htb", [TT, P, HCH * TW], bf16)        # chunk-major partition rows
    htb8 = dp("htb8", [TT, P, HCH * TW], bf16)      # hidden/8, chunk-major rows
    h64c3 = dp("h64c3", [P, HCH * TW], bf16)        # hidden/64, chunk 3 only
    htsh = dp("htsh", [P, HCH * SH], bf16)          # this core's kv_a token shard
    wq = dp("wq", [P, HCH * HPC * DQK], bf16)
    wkva = dp("wkva", [P, (KVC + 1) * HCH * P], bf16)   # f-major blocks
    wkvbn = dp("wkvbn", [P, KVC * HPC * DN], bf16)
    wkvbv = dp("wkvbv", [P, KVC * HPC * DV], bf16)
    wo = dp("wo", [P, HPC * HID], bf16)             # pre-divided by 8
    wg = dp("wg", [P, HCH * FPC], bf16)
    wu = dp("wu", [P, HCH * FPC], bf16)
    wd = dp("wd", [P, (FPC // P) * HID], bf16)
    cosf = dp("cosf", [P, T], bf16)      # rows 64:128 duplicate 0:64
    sinf = dp("sinf", [P, T], bf16)
    cossh = dp("cossh", [DR, SH], bf16)
    sinsh = dp("sinsh", [DR, SH], bf16)
    masks = dp("masks", [P, TW + 384], bf16)        # shifted-window causal mask
    out = nc.dram_tensor("o", [TT, HID // NC_N, TW], bf16, kind="ExternalOutput")
    rg = [list(range(NC_N))]

    with tile.TileContext(nc) as tc:
        with tc.tile_pool(name="const", bufs=1) as cpool, \
             tc.tile_pool(name="dram", bufs=1, space="DRAM") as dram, \
             tc.tile_pool(name="ps", bufs=1, space="PSUM") as ps, \
             tc.tile_pool(name="wrk", bufs=3) as wrk, \
             tc.tile_pool(name="row", bufs=2) as row:
            ones_col = cpool.tile([P, 1], bf16)
            nc.vector.memset(ones_col[:], 1.0)
            ones_row = cpool.tile([1, P], bf16)
            nc.vector.memset(ones_row[:], 1.0)
            epsb = cpool.tile([P, 1], f32)
            nc.vector.memset(epsb[:], EPS)
            eps64 = cpool.tile([P, 1], f32)
            nc.vector.memset(eps64[:], EPS / 64.0)

            ag_in = dram.tile([P, AGW], bf16, name="ag_in")
            ag_out = dram.tile([NC_N, P, AGW], bf16, addr_space="Shared",
                               name="ag_out")
            ar_in = [dram.tile([HID, TW], bf16, name=f"ar_in{t}") for t in range(TT)]
            ar_out = [dram.tile([HID, TW], bf16, addr_space="Shared",
                                name=f"ar_out{t}") for t in range(TT)]
            rs_in = [dram.tile([HID, TW], bf16, name=f"rs_in{t}")
                     for t in range(TT)]
            rs_out = [dram.tile([HID // NC_N, TW], bf16, name=f"rs_out{t}")
                      for t in range(TT)]

            # helper: [1,W] f32 PSUM sumsq row -> wide f32 inv-scale [P,W]
            def inv_chain(ssp, w, scale, bias, name, out=None):
                nrow = row.tile([1, TW], bf16, tag="nrow", bufs=1, name=f"nr_{name}")
                nc.scalar.copy(nrow[:, :w], ssp[:, :w])
                bcp = ps.tile([P, TW], f32, tag="big", bufs=4, name=f"bc_{name}")
                nc.tensor.matmul(bcp[:, :w], ones_row[:], nrow[:, :w],
                                 start=True, stop=True)
                sd = wrk.tile([P, TW], f32, tag="sd", bufs=1, name=f"sd_{name}")
                nc.scalar.activation(sd[:, :w], bcp[:, :w], AF.Sqrt,
                                     bias=bias, scale=scale)
                if out is None:
                    out = wrk.tile([P, TW], f32, tag="inv", bufs=2,
                                   name=f"inv_{name}")[:, :w]
                nc.vector.reciprocal(out, sd[:, :w])
                return out

            with tc.tile_pool(name="pers", bufs=1) as pers:
                qsb = pers.tile([P, 3, T], bf16)           # 12K
                qrope = pers.tile([P, T], bf16)            # head h at rows h*64
                krope = pers.tile([P, T], bf16)            # both halves identical

                # ---- A0: kv_a shard + AllGather (fires early) ----
                with tc.tile_pool(name="shp", bufs=1) as shp:
                    hts = shp.tile([P, HCH, SH], bf16)
                    nc.sync.dma_start(hts[:], htsh.ap())
                    wkva_sb = shp.tile([P, KVC + 1, HCH, P], bf16)   # 20K
                    for f in range(KVC + 1):
                        nc.sync.dma_start(wkva_sb[:, f, :, :],
                                          wkva.ap()[:, f * HCH * P:(f + 1) * HCH * P])
                    css = shp.tile([DR, SH], bf16)
                    nc.sync.dma_start(css[:], cossh.ap())
                    sns = shp.tile([DR, SH], bf16)
                    nc.sync.dma_start(sns[:], sinsh.ap())

                    # shard input-norm scale r1_sh
                    sshp = ps.tile([1, TW], f32, tag="r", bufs=2, name="sshp")
                    for o in range(HCH):
                        sqs = wrk.tile([P, TW], bf16, tag="sq", bufs=2, name="sqs")
                        nc.scalar.square(sqs[:, :SH], hts[:, o, :])
                        nc.tensor.matmul(sshp[:, :SH], ones_col[:], sqs[:, :SH],
                                         start=(o == 0), stop=(o == HCH - 1))
                    rsh = inv_chain(sshp, SH, 1.0 / HID, epsb[:], "rsh")

                    # latent projection for the shard
                    lats = shp.tile([P, KVC * SH], bf16)
                    kpes = shp.tile([DR, SH], bf16)
                    ss2p = ps.tile([1, TW], f32, tag="r", bufs=2, name="ss2p")
                    for f in range(KVC + 1):
                        wid = P if f < KVC else DR
                        lp = ps.tile([P, TW], f32, tag="big", bufs=4, name="lp")
                        for o in range(HCH):
                            nc.tensor.matmul(lp[:wid, :SH],
                                             wkva_sb[:, f, o, :wid],
                                             hts[:, o, :],
                                             start=(o == 0), stop=(o == HCH - 1))
                        if f < KVC:
                            nc.vector.tensor_copy(out=lats[:, f * SH:(f + 1) * SH],
                                                  in_=lp[:, :SH])
                            sq2 = wrk.tile([P, TW], bf16, tag="sq", bufs=2, name="sq2")
                            nc.scalar.square(sq2[:, :SH], lp[:, :SH])
                            nc.tensor.matmul(ss2p[:, :SH], ones_col[:], sq2[:, :SH],
                                             start=(f == 0), stop=(f == KVC - 1))
                        else:
                            nc.vector.tensor_tensor(kpes[:], lp[:DR, :SH],
                                                    rsh[:DR, :SH], MUL)
                    r2sh = inv_chain(ss2p, SH, 1.0 / KVR, epsb[:], "r2sh")
                    kvas = shp.tile([P, KVC * SH], bf16)
                    for f in range(KVC):
                        nc.vector.tensor_tensor(kvas[:, f * SH:(f + 1) * SH],
                                                lats[:, f * SH:(f + 1) * SH],
                                                r2sh[:, :SH], MUL)
                    # rope k_pe shard: [x1(32); x2(32)] layout
                    ksw = wrk.tile([DR, SH], bf16, tag="rps", bufs=1, name="ksw")
                    nc.sync.dma_start(ksw[0:32, :], kpes[32:64, :])
                    nc.sync.dma_start(ksw[32:64, :], kpes[0:32, :])
                    ktmp = wrk.tile([DR, SH], bf16, tag="rps", bufs=1, name="ktmp")
                    nc.vector.tensor_tensor(ktmp[:], kpes[:], css[:], MUL)
                    krs = shp.tile([DR, SH], bf16)
                    nc.vector.tensor_tensor(krs[:], ksw[:], sns[:], MUL)
                    nc.vector.tensor_tensor(krs[:], krs[:], ktmp[:], ADD)
                    # pack shard -> ag_in and AllGather
                    nc.sync.dma_start(ag_in[:, 0:KVC * SH], kvas[:])
                    nc.sync.dma_start(ag_in[0:DR, KVC * SH:KVC * SH + SH], krs[:])
                    nc.gpsimd.collective_compute(
                        "AllGather", mybir.AluOpType.bypass, ins=[ag_in[:].opt()],
                        outs=[ag_out[:].opt()], replica_groups=rg)

                # ---- A1: full input norm + q projection + q rope ----
                with tc.tile_pool(name="a1", bufs=1) as a1:
                    ht = a1.tile([P, TT, HCH, TW], bf16)         # 64K
                    for g in range(4):
                        nc.sync.dma_start(
                            ht[:, 0, 4 * g:4 * g + 4, :],
                            htb.ap()[0, :, 4 * g * TW:(4 * g + 4) * TW])
                    wq_sb = a1.tile([P, HCH, HPC * DQK], bf16)   # 12K
                    nc.sync.dma_start(wq_sb[:], wq.ap())
                    for t in range(1, TT):
                        for g in range(4):
                            nc.sync.dma_start(
                                ht[:, t, 4 * g:4 * g + 4, :],
                                htb.ap()[t, :, 4 * g * TW:(4 * g + 4) * TW])
                    cs = a1.tile([P, T], bf16)
                    nc.sync.dma_start(cs[:], cosf.ap())
                    sn = a1.tile([P, T], bf16)
                    nc.sync.dma_start(sn[:], sinf.ap())
                    bc1 = a1.tile([P, TT, TW], f32)              # 8K

                    for t in range(TT):
                        ssp = ps.tile([1, TW], f32, tag="r", bufs=2, name="ssp")
                        for o in range(HCH):
                            sq = wrk.tile([P, TW], bf16, tag="sq", bufs=2, name="sq")
                            nc.scalar.square(sq[:], ht[:, t, o, :])
                            nc.tensor.matmul(ssp[:], ones_col[:], sq[:],
                                             start=(o == 0), stop=(o == HCH - 1))
                        inv_chain(ssp, TW, 1.0 / HID, epsb[:], f"r1_{t}",
                                  out=bc1[:, t, :])
                        # q projection for this chunk (SCALING folded into wq)
                        for f in range(3):
                            qp = ps.tile([P, TW], f32, tag="big", bufs=4, name="qp")
                            for o in range(HCH):
                                nc.tensor.matmul(qp[:], wq_sb[:, o, f * P:(f + 1) * P],
                                                 ht[:, t, o, :],
                                                 start=(o == 0), stop=(o == HCH - 1))
                            nc.vector.tensor_tensor(qsb[:, f, t * TW:(t + 1) * TW],
                                                    qp[:], bc1[:, t, :], MUL)

                    # q rope: [x1(32); x2(32)] per head, head h on rows h*64
                    src = qsb[:, 2, :]
                    sw = a1.tile([P, T], bf16, tag="rope", bufs=2, name="qsw")
                    for h in range(HPC):
                        nc.sync.dma_start(sw[h * DR:h * DR + 32, :],
                                          src[h * DR + 32:h * DR + 64, :])
                        nc.sync.dma_start(sw[h * DR + 32:h * DR + 64, :],
                                          src[h * DR:h * DR + 32, :])
                    tmp = a1.tile([P, T], bf16, tag="rope", bufs=2, name="qtmp")
                    nc.vector.tensor_tensor(tmp[:], src, cs[:], MUL)
                    nc.vector.tensor_tensor(qrope[:], sw[:], sn[:], MUL)
                    nc.vector.tensor_tensor(qrope[:], qrope[:], tmp[:], ADD)
                    # k rope full comes pre-roped from the AllGather (both halves)
                    for c in range(NC_N):
                        for h in range(HPC):
                            nc.sync.dma_start(
                                krope[h * DR:(h + 1) * DR, c * SH:(c + 1) * SH],
                                ag_out[c, 0:DR, KVC * SH:KVC * SH + SH])

                # ---- bw: MLP weights (DMAs issued after attention inputs) ----
                with tc.tile_pool(name="bw", bufs=1) as bw:
                    wg_sb = bw.tile([P, HCH, FPC], bf16)       # 32K
                    wu_sb = bw.tile([P, HCH, FPC], bf16)       # 32K
                    wd_sb = bw.tile([P, FPC // P, HID], bf16)  # 16K

                    with tc.tile_pool(name="att", bufs=1) as att:
                        knope = att.tile([P, HPC, T], bf16)        # 8K
                        vnat = att.tile([P, T // P, HPC * DV], bf16)  # 8K
                        wo_sb = att.tile([P, HPC, HID], bf16)      # 8K
                        msk = att.tile([P, TW + 384], bf16)        # 1.75K
                        nc.sync.dma_start(msk[:], masks.ap())
                        nc.sync.dma_start(wo_sb[:], wo.ap())

                        with tc.tile_pool(name="kvp", bufs=1) as kvp:
                            # kva laid out core-major so each load is contiguous
                            kva = kvp.tile([P, NC_N, KVC, SH], bf16)   # 16K
                            for c in range(NC_N):
                                nc.sync.dma_start(kva[:, c, :, :],
                                                  ag_out[c, :, 0:KVC * SH])
                            wkvbn_sb = kvp.tile([P, KVC, HPC * DN], bf16)
                            nc.sync.dma_start(wkvbn_sb[:], wkvbn.ap())
                            wkvbv_sb = kvp.tile([P, KVC, HPC * DV], bf16)
                            nc.sync.dma_start(wkvbv_sb[:], wkvbv.ap())
                            # now queue the MLP weight prefetch behind these
                            nc.sync.dma_start(wg_sb[:], wg.ap())
                            nc.sync.dma_start(wu_sb[:], wu.ap())
                            nc.sync.dma_start(wd_sb[:], wd.ap())

                            for h in range(HPC):
                                for t in range(TT):
                                    kp = ps.tile([P, TW], f32, tag="big", bufs=4,
                                                 name="kp")
                                    for c in range(KVC):
                                        nc.tensor.matmul(
                                            kp[:], wkvbn_sb[:, c, h * P:(h + 1) * P],
                                            kva[:, 2 * t:2 * t + 2, c, :],
                                            start=(c == 0), stop=(c == KVC - 1))
                                    nc.vector.tensor_copy(
                                        out=knope[:, h, t * TW:(t + 1) * TW], in_=kp[:])
                            for to in range(T // P):
                                vp = ps.tile([P, TW], f32, tag="att", bufs=2,
                                             name="vp")
                                co, po = to // 2, (to % 2) * P
                                for c in range(KVC):
                                    nc.tensor.matmul(
                                        vp[:, :HPC * DV],
                                        kva[:, co, c, po:po + P],
                                        wkvbv_sb[:, c, :],
                                        start=(c == 0), stop=(c == KVC - 1))
                                nc.vector.tensor_copy(out=vnat[:, to, :],
                                                      in_=vp[:, :HPC * DV])

                        # B-chunk tiles live in the space kvp just freed
                        with tc.tile_pool(name="bp", bufs=1) as bp:
                            h2s = [None] * TT
                            acts = [None] * TT

                            def b_pre(t):
                                # x/8 = hidden/8 + ar_out[t] (attn/8); h2 doubles
                                # as the x/8 store. Fat chunk DMA, no AR dep.
                                h2 = bp.tile([P, HCH, TW], bf16, tag="h2", bufs=2,
                                             name=f"h2_{t}")
                                for g in range(4):
                                    nc.sync.dma_start(
                                        h2[:, 4 * g:4 * g + 4, :],
                                        htb8.ap()[t, :, 4 * g * TW:(4 * g + 4) * TW])
                                h2s[t] = h2

                            def b_pre3():
                                h2s[3] = bp.tile([P, HCH, TW], bf16, tag="h2",
                                                 bufs=2, name="h2_3")

                            def b_fin(t, folded=False):
                                # folded: ar_out[t] already holds x/8
                                h2 = h2s[t]
                                ssp3 = ps.tile([1, TW], f32, tag="r", bufs=2,
                                               name="ssp3")
                                for o in range(HCH):
                                    if folded:
                                        nc.sync.dma_start(
                                            h2[:, o, :],
                                            ar_out[t][o * P:(o + 1) * P, :])
                                    else:
                                        aro = wrk.tile([P, TW], bf16, tag="h64",
                                                       bufs=3, name="aro")
                                        nc.sync.dma_start(
                                            aro[:], ar_out[t][o * P:(o + 1) * P, :])
                                        nc.vector.tensor_tensor(h2[:, o, :],
                                                                h2[:, o, :],
                                                                aro[:], ADD)
                                    sq3 = wrk.tile([P, TW], bf16, tag="sq", bufs=2,
                                                   name="sq3")
                                    nc.scalar.square(sq3[:], h2[:, o, :])
                                    nc.tensor.matmul(ssp3[:], ones_col[:], sq3[:],
                                                     start=(o == 0),
                                                     stop=(o == HCH - 1))
                                # bc3' = 8/sqrt(ms+eps) = 1/sqrt(ss8/HID + eps/64)
                                bc3 = inv_chain(ssp3, TW, 1.0 / HID, eps64[:],
                                                f"b3_{t}")
                                for o in range(HCH):
                                    nc.vector.tensor_tensor(h2[:, o, :], h2[:, o, :],
                                                            bc3, MUL)

                            def attn_chunk(b, qt, fold=False):
                                tt = b * QT + qt
                                qc0 = b * S + qt * TW
                                nkt = 4 * qt + 4
                                dnp = [ps.tile([1, TW], f32, tag="r", bufs=2,
                                               name=f"dnp{h}") for h in range(HPC)]
                                atp = [ps.tile([P, TW], f32, tag="att", bufs=2,
                                               name=f"atp{h}") for h in range(HPC)]
                                exs = [[None] * nkt for _ in range(HPC)]

                                def consume(h, kt):
                                    nc.tensor.matmul(dnp[h][:], ones_col[:],
                                                     exs[h][kt][:],
                                                     start=(kt == 0),
                                                     stop=(kt == nkt - 1))
                                    nc.tensor.matmul(atp[h][:],
                                                     vnat[:, b * KT + kt,
                                                          h * DV:(h + 1) * DV],
                                                     exs[h][kt][:],
                                                     start=(kt == 0),
                                                     stop=(kt == nkt - 1))

                                # both heads interleaved: 4 independent tiles in
                                # flight keep the scores->mask->exp->consume chain
                                # off the PE critical path
                                for kt in range(nkt):
                                    kc0 = b * S + kt * P
                                    j = kt - 4 * qt
                                    for h in range(HPC):
                                        scp = ps.tile([P, TW], f32, tag="big",
                                                      bufs=4, name="scp")
                                        nc.tensor.matmul(scp[:],
                                                         knope[:, h, kc0:kc0 + P],
                                                         qsb[:, h, qc0:qc0 + TW],
                                                         start=True, stop=False)
                                        nc.tensor.matmul(
                                            scp[:],
                                            krope[h * DR:(h + 1) * DR, kc0:kc0 + P],
                                            qrope[h * DR:(h + 1) * DR, qc0:qc0 + TW],
                                            start=False, stop=True)
                                        ex = wrk.tile([P, TW], bf16, tag="ex",
                                                      bufs=6, name="ex")
                                        if j >= 0:
                                            mtmp = wrk.tile([P, TW], f32, tag="mt",
                                                            bufs=2, name="mtmp")
                                            m0 = 384 - j * P
                                            nc.vector.tensor_tensor(
                                                mtmp[:], scp[:],
                                                msk[:, m0:m0 + TW], ADD)
                                            nc.scalar.activation(ex[:], mtmp[:],
                                                                 AF.Exp)
                                        else:
                                            mtmp = wrk.tile([P, TW], f32, tag="mt",
                                                            bufs=2, name="mtmp0")
                                            nc.vector.tensor_scalar_add(
                                                mtmp[:], scp[:], 0.0)
                                            nc.scalar.activation(ex[:], mtmp[:],
                                                                 AF.Exp)
                                        exs[h][kt] = ex
                                    if kt >= 2:
                                        for h in range(HPC):
                                            consume(h, kt - 2)
                                for h in range(HPC):
                                    consume(h, max(nkt - 2, 0))
                                for h in range(HPC):
                                    if nkt > 1:
                                        consume(h, nkt - 1)
                                # 1/denom: narrow copy -> PE bcast -> wide recip
                                atns = []
                                dbcs = []
                                for h in range(HPC):
                                    drow = row.tile([1, TW], bf16, tag="nrow",
                                                    bufs=1, name="drow")
                                    nc.scalar.copy(drow[:], dnp[h][:])
                                    dbp = ps.tile([P, TW], f32, tag="big", bufs=4,
                                                  name="dbp")
                                    nc.tensor.matmul(dbp[:], ones_row[:], drow[:],
                                                     start=True, stop=True)
                                    dbc = wrk.tile([P, TW], f32, tag="inv", bufs=2,
                                                   name="dbc")
                                    nc.vector.reciprocal(dbc[:], dbp[:])
                                    dbcs.append(dbc)
                                for h in range(HPC):
                                    atn = att.tile([P, TW], bf16, tag="atn", bufs=2,
                                                   name="atn")
                                    nc.vector.tensor_tensor(atn[:], atp[h][:],
                                                            dbcs[h][:], MUL)
                                    atns.append(atn)
                                atn0, atn = atns
                                # o_proj partial (wo/8 folded) + hidden/64 -> x/8
                                for ho in range(HCH):
                                    op = ps.tile([P, TW], f32, tag="big", bufs=4,
                                                 name="op")
                                    nc.tensor.matmul(op[:],
                                                     wo_sb[:, 0, ho * P:(ho + 1) * P],
                                                     atn0[:], start=True, stop=False)
                                    nc.tensor.matmul(op[:],
                                                     wo_sb[:, 1, ho * P:(ho + 1) * P],
                                                     atn[:], start=False, stop=True)
                                    osb = wrk.tile([P, TW], bf16, tag="ex", bufs=6,
                                                   name="osb")
                                    if fold:
                                        h64 = wrk.tile([P, TW], bf16, tag="h64",
                                                       bufs=3, name="h64")
                                        nc.sync.dma_start(
                                            h64[:],
                                            h64c3.ap()[:, ho * TW:(ho + 1) * TW])
                                        nc.vector.tensor_tensor(osb[:], op[:],
                                                                h64[:], ADD)
                                    else:
                                        nc.vector.tensor_copy(out=osb[:], in_=op[:])
                                    nc.sync.dma_start(
                                        ar_in[tt][ho * P:(ho + 1) * P, :], osb[:])
                                nc.gpsimd.collective_compute(
                                    "AllReduce", ADD, ins=[ar_in[tt][:].opt()],
                                    outs=[ar_out[tt][:].opt()], replica_groups=rg)

                            def b_gateup(t):
                                h2 = h2s[t]
                                act = bp.tile([P, FPC // P, TW], bf16, tag="act",
                                              bufs=1, name=f"act_{t}")
                                for fi in range(FPC // P):
                                    gp = ps.tile([P, TW], f32, tag="big", bufs=4,
                                                 name="gp")
                                    for o in range(HCH):
                                        nc.tensor.matmul(
                                            gp[:], wg_sb[:, o, fi * P:(fi + 1) * P],
                                            h2[:, o, :],
                                            start=(o == 0), stop=(o == HCH - 1))
                                    up = ps.tile([P, TW], f32, tag="att", bufs=2,
                                                 name="up")
                                    for o in range(HCH):
                                        nc.tensor.matmul(
                                            up[:], wu_sb[:, o, fi * P:(fi + 1) * P],
                                            h2[:, o, :],
                                            start=(o == 0), stop=(o == HCH - 1))
                                    gs = wrk.tile([P, TW], bf16, tag="gs", bufs=1,
                                                  name="gs")
                                    nc.scalar.activation(gs[:], gp[:], AF.Silu)
                                    nc.vector.tensor_tensor(act[:, fi, :], up[:],
                                                            gs[:], MUL)
                                acts[t] = act

                            def b_down(t, folded=False):
                                act = acts[t]
                                for ho in range(HCH):
                                    dpp = ps.tile([P, TW], f32, tag="big",
                                                  bufs=4, name="dpp")
                                    for c in range(FPC // P):
                                        nc.tensor.matmul(
                                            dpp[:],
                                            wd_sb[:, c, ho * P:(ho + 1) * P],
                                            act[:, c, :],
                                            start=(c == 0),
                                            stop=(c == FPC // P - 1))
                                    xo = wrk.tile([P, TW], bf16, tag="h64",
                                                  bufs=3, name="xo")
                                    nc.sync.dma_start(
                                        xo[:], ar_out[t][ho * P:(ho + 1) * P, :])
                                    dsb = wrk.tile([P, TW], bf16, tag="ex",
                                                   bufs=6, name="dsb")
                                    if folded:
                                        nc.vector.tensor_tensor(dsb[:], dpp[:],
                                                                xo[:], ADD)
                                    else:
                                        xh = wrk.tile([P, TW], bf16, tag="h64",
                                                      bufs=3, name="xh")
                                        nc.sync.dma_start(
                                            xh[:],
                                            htb8.ap()[t, :, ho * TW:(ho + 1) * TW])
                                        nc.vector.tensor_tensor(dsb[:], dpp[:],
                                                                xo[:], ADD)
                                        nc.vector.tensor_tensor(dsb[:], dsb[:],
                                                                xh[:], ADD)
                                    nc.sync.dma_start(
                                        rs_in[t][ho * P:(ho + 1) * P, :], dsb[:])
                                nc.gpsimd.collective_compute(
                                    "ReduceScatter", ADD, ins=[rs_in[t][:].opt()],
                                    outs=[rs_out[t][:].opt()], replica_groups=rg)
                                nc.sync.dma_start(out.ap()[t], rs_out[t][:])

                            attn_chunk(0, 0)
                            b_pre(0)
                            attn_chunk(0, 1)
                            b_pre(1)
                            attn_chunk(1, 0)
                            attn_chunk(1, 1)
                            b_fin(0)
                            b_gateup(0)
                            b_fin(1)
                            b_pre(2)
                            b_down(0)
                            b_gateup(1)
                            b_fin(2)
                            b_pre(3)
                            b_down(1)
                            b_gateup(2)
                            b_fin(3)
                            b_down(2)
                            b_gateup(3)
                            b_down(3)
    nc.compile()
    return nc


def _row_major(w, blocks, width):
    # [blocks*P, width] -> [P, blocks*width] partition rows
    return np.ascontiguousarray(
        w.reshape(blocks, P, width).transpose(1, 0, 2).reshape(P, blocks * width))


def _prep(hidden_states, positions, w_in_ln, w_q, w_kv_a, w_kv_a_ln,
          w_kv_b, w_o, w_post_ln, w_gate, w_up, w_down):
    hT = np.ascontiguousarray(
        np.asarray(hidden_states, np.float32).reshape(T, HID).T)

    pos = np.asarray(positions).reshape(-1).astype(np.float64)
    inv = ROPE_BASE ** (-np.arange(0, DR, 2, dtype=np.float64) / DR)
    fr = pos[:, None] * inv[None, :]                      # [T, 32]
    c32 = np.cos(fr).T.astype(np.float32)                 # [32, T]
    s32 = np.sin(fr).T.astype(np.float32)
    cosf = np.concatenate([c32, c32, c32, c32], 0)        # [128, T], dup halves
    sinf = np.concatenate([-s32, s32, -s32, s32], 0)

    r = np.arange(P)[:, None]
    u = np.arange(TW + 384)[None, :]
    masks = np.where(u >= r + 384, 0.0, NEG).astype(np.float32)   # [128, 896]

    w_in_ln = np.asarray(w_in_ln, np.float32)
    wqf = (np.asarray(w_q, np.float32) * w_in_ln[:, None] * SCALING
           ).reshape(HID, H, DQK)
    wkvaf = np.asarray(w_kv_a, np.float32) * w_in_ln[:, None]
    kpe_w = wkvaf[:, KVR:]
    wkva_p = np.concatenate([wkvaf[:, :KVR], kpe_w[:, 0::2], kpe_w[:, 1::2]], 1)
    wkvbf = (np.asarray(w_kv_b, np.float32)
             * np.asarray(w_kv_a_ln, np.float32)[:, None]).reshape(KVR, H, DN + DV)
    w_post_ln = np.asarray(w_post_ln, np.float32)
    wgf = np.asarray(w_gate, np.float32) * w_post_ln[:, None]
    wuf = np.asarray(w_up, np.float32) * w_post_ln[:, None]
    wdf = np.asarray(w_down, np.float32)
    wof = (np.asarray(w_o, np.float32) / NC_N).reshape(H, DV, HID)

    htb = hT.astype(BF)
    # chunk-major partition rows: [t, p, o*TW+w] = hT[o*128+p, t*TW+w]
    def chunk_major(a):
        return np.ascontiguousarray(
            a.reshape(HCH, P, TT, TW).transpose(2, 1, 0, 3).reshape(TT, P, HCH * TW))
    htb4 = chunk_major(htb)
    htb8 = chunk_major((hT / 8.0).astype(BF))
    h64c3 = np.ascontiguousarray(chunk_major((hT / 64.0).astype(BF))[TT - 1])

    in_maps = []
    for core in range(NC_N):
        hs = [2 * core, 2 * core + 1]
        nopes = np.concatenate([wqf[:, h, :DN] for h in hs], 1)
        pes = []
        for h in hs:
            pe = wqf[:, h, DN:]
            pes += [pe[:, 0::2], pe[:, 1::2]]
        wq_c = np.concatenate([nopes] + pes, 1)
        c0 = core * SH
        in_maps.append({
            "htb": htb4,
            "htb8": htb8,
            "h64c3": h64c3,
            "htsh": _row_major(np.ascontiguousarray(htb[:, c0:c0 + SH]).astype(
                np.float32), HCH, SH).astype(BF),
            "wq": _row_major(wq_c, HCH, HPC * DQK).astype(BF),
            "wkva": np.ascontiguousarray(
                np.pad(wkva_p, ((0, 0), (0, (KVC + 1) * P - (KVR + DR))))
                .reshape(HCH, P, KVC + 1, P).transpose(1, 2, 0, 3)
                .reshape(P, (KVC + 1) * HCH * P)).astype(BF),
            "wkvbn": _row_major(
                np.concatenate([wkvbf[:, h, :DN] for h in hs], 1),
                KVC, HPC * DN).astype(BF),
            "wkvbv": _row_major(
                np.concatenate([wkvbf[:, h, DN:] for h in hs], 1),
                KVC, HPC * DV).astype(BF),
            "wo": _row_major(np.concatenate([wof[h] for h in hs], 0),
                             HPC, HID).astype(BF),
            "wg": _row_major(wgf[:, core * FPC:(core + 1) * FPC],
                             HCH, FPC).astype(BF),
            "wu": _row_major(wuf[:, core * FPC:(core + 1) * FPC],
                             HCH, FPC).astype(BF),
            "wd": _row_major(wdf[core * FPC:(core + 1) * FPC, :],
                             FPC // P, HID).astype(BF),
            "cosf": cosf.astype(BF),
            "sinf": sinf.astype(BF),
            "cossh": cosf[0:DR, c0:c0 + SH].astype(BF).copy(),
            "sinsh": sinf[0:DR, c0:c0 + SH].astype(BF).copy(),
            "masks": masks.astype(BF),
        })
    return in_maps


def kernel(**inputs):
    if "nc" not in _CACHE:
        _CACHE["nc"] = _build()
    nc = _CACHE["nc"]
    in_maps = _prep(**inputs)
    res = run_bass_kernel_spmd(nc, in_maps, core_ids=list(range(NC_N)))
    # o: per-core [TT, HID//NC_N, TW] bf16 -> full [HID, T] f32
    outT = np.concatenate(
        [np.concatenate(list(res.results[c]["o"].astype(np.float32)), axis=1)
         for c in range(NC_N)], 0)
    return np.ascontiguousarray(outT.T).reshape(B, S, HID).astype(np.float32)
